# revision 80
# baseline (speedup 1.0000x reference)
"""Trainium2 Bass kernel for MultiHeadAttention + residual + BatchNorm.

Model (reference):
  q = query @ Wq.T ; k = key @ Wk.T ; v = key @ Wv.T    (per-head split)
  score = q k^T / sqrt(D), causal mask, softmax over keys
  res   = (attn @ v) + query
  out   = batchnorm(res over all (N*L) rows, per feature) * gamma + beta

Sharding over 8 cores: FEATURE sharding. Core c owns heads {2c, 2c+1}
(features [128c, 128c+128)) for ALL batches. BatchNorm statistics are
then core-local, so no collective is needed at all.

All-fp8 attention datapath: projections use fp8 DoubleRow; projected
q/k are re-quantized to fp8 and DMA-shuffled into a [32-part, 2-pair]
layout (the host permutes Wq/Wk columns so each shuffle is 2 full-width
DMAs) so the score matmuls also run as fp8 DoubleRow (half the PE
columns).  Attention weights are produced in fp8 by ACT (exp writes
fp8 directly) and DVE (e4m3 *bits* via an int8 Schraudolph
scalar_tensor_tensor - valid because |score*scale| is small so the
biased-exponent byte is always in [38, 90]).  Full key blocks are
exp'd pairwise into one [128, 2048] fp8 tile so the attention@V
matmuls run as fp8 DoubleRow over 256 keys at a time.  V carries an
appended constant column whose products accumulate into per-(ic,n,h,S)
columns of a once-cleared PSUM bank, giving softmax denominators
without extra PE passes.

Engine budget: scores/AV/projections on PE; exp split ACT/DVE by a
greedy cost balancer; masks + residual-adds on GPSIMD (which cannot
touch PSUM); DVE carries the PSUM-coupled glue (re-quantize copies,
normalize, reciprocal, batch-norm).  PSUM is exactly packed: 6 score
half-slots (shared with projection accumulators) + attention-value
bank + denominator/stats bank.
"""

import math
import sys

sys.path.insert(0, "/opt/trn_rl_repo")

import numpy as np
import ml_dtypes

import concourse.bass as bass
import concourse.mybir as mybir
from concourse import bacc
import concourse.tile as tile

F32 = mybir.dt.float32
BF16 = mybir.dt.bfloat16
FP8 = mybir.dt.float8e4
I8 = mybir.dt.int8
BF16_NP = ml_dtypes.bfloat16
FP8_NP = mybir.dt.np(FP8)
# q/k weights are scaled by 16 on the host so fp8e4 stays out of the
# subnormal range.
W8_SCALE = 16.0

N = 4
L = 2048
D = 1024
H = 16
P = 64
NCORES = 8
FC = D // NCORES       # features per core = 128
H2 = 2                 # heads per core
EPS = 1e-5
SCALE = 1.0 / math.sqrt(D)
NL = N * L             # 8192 rows in the global batch norm

# fp8 staging scales: q8 = (16 q) * SQK = 8 q ; k8 likewise ; v8 = 8 v.
SQK = 0.5
SV = 8.0
VONES = 8.0            # denominator column value (folded out by 1/(8*sum))
# score PSUM = sum q8 k8 = 64 * raw score -> exp argument scale:
EXP_SCALE = SCALE / 64.0
# e4m3-bit Schraudolph: i8 = round(st2 * A8V + B8V) gives bits of
# ~exp(st2 * EXP_SCALE)
A8V = (8.0 / math.log(2.0)) * EXP_SCALE
B8V = 56.0 - 0.3494 + 0.5

# static per-unit engine-busy costs (ns) for the greedy load balancer.
# NOTE: the Pool/GPSIMD engine cannot access PSUM, so only SBUF-input
# work (masks, the drain residual-add) is Pool-eligible.
COST_FULL_HALF = {"A": 612, "D": 631}
COST_DIAG_HALF = [{"A": 612, "D": 631}, {"A": 505, "D": 504},
                  {"A": 398, "D": 376}, {"A": 292, "D": 247}]
COST_MASK = {"D": 1127, "P": 2126}      # merged, one per (ic, n)
COST_DRAIN = {"D": 523}                 # recip + normalize TT (PSUM: DVE)
COST_DRAIN_ADD = {"D": 127, "P": 603}   # residual add (SBUF)
COST_DSQ = {"D": 193, "P": 1110}        # res^2 for the variance (SBUF)
COST_VCOPY = {"A": 292, "D": 247}
COST_QKCOPY = {"D": 631, "A": 668}

# software-pipeline stage lags
EXP_LAG = 1
MASK_LAG = 4
AV_LAG = 8
DEBUG_NOLAG = False

_cached = {}


def build_program(l=L):
    """Build the SPMD Bass program (identical on all 8 cores)."""
    nc = bacc.Bacc("TRN2", target_bir_lowering=False, debug=False,
                   num_devices=NCORES)

    ic_n = l // 512        # 512-row query chunks
    nlc = N * l            # rows per core (all batches)

    xq8_nd = nc.dram_tensor("xq8_nd", [D, nlc], FP8,
                            kind="ExternalInput").ap()
    xk8_nd = nc.dram_tensor("xk8_nd", [D, nlc], FP8,
                            kind="ExternalInput").ap()
    wq8 = nc.dram_tensor("wq8", [D, FC], FP8, kind="ExternalInput").ap()
    wk8 = nc.dram_tensor("wk8", [D, FC], FP8, kind="ExternalInput").ap()
    wvt = nc.dram_tensor("wvt", [D, FC], FP8, kind="ExternalInput").ap()
    # residual input, pre-blocked on the host to the SBUF res layout:
    # [row-in-block, (n, ic*4+S, f)]
    q_res = nc.dram_tensor("q_res", [128, (nlc // 128) * FC], BF16,
                           kind="ExternalInput").ap()
    gamma = nc.dram_tensor("gamma", [1, FC], F32, kind="ExternalInput").ap()
    beta = nc.dram_tensor("beta", [1, FC], F32, kind="ExternalInput").ap()
    # output keeps the SBUF res layout: [row-in-block, (n, ic, S, f)] so the
    # store is one fat contiguous DMA per batch; the host untangles it.
    out_s = nc.dram_tensor("out_s", [128, (nlc // 128) * FC], BF16,
                           kind="ExternalOutput").ap()

    jblocks = l // 128     # 16 key blocks per batch
    nls = nlc // 128       # 64 ls blocks of res

    from contextlib import ExitStack
    with ExitStack() as stack:
        tc = stack.enter_context(tile.TileContext(nc))
        pool = {}
        for nm, bufs, space in (
                ("consts", 1, None), ("persist", 1, None), ("wt", 1, None),
                ("qtp", 2, None), ("xq", 6, None), ("xk8", 6, None),
                ("qf8", 3, None), ("kf8", 3, None),
                ("at8", 12, None), ("at8d", 3, None),
                ("sq", 4, None),
                ("outp", 2, None), ("bnp", 1, None), ("small", 10, None),
                ("st2", 6, "PSUM"), ("av", 1, "PSUM"),
                ("stat", 1, "PSUM")):
            kw = {"name": nm, "bufs": bufs}
            if space:
                kw["space"] = space
            pool[nm] = stack.enter_context(tc.tile_pool(**kw))
        consts, persist, wtp = pool["consts"], pool["persist"], pool["wt"]
        qtp, xqp, xk8p = pool["qtp"], pool["xq"], pool["xk8"]
        qf8p, kf8p = pool["qf8"], pool["kf8"]
        at8p, at8dp = pool["at8"], pool["at8d"]
        sqp = pool["sq"]
        outp, bnp, smallp = pool["outp"], pool["bnp"], pool["small"]
        st2p, avp, statp = pool["st2"], pool["av"], pool["stat"]

        # greedy engine balancer: assign each elementwise unit to the
        # engine with the least projected busy time.  Hard-assigned work
        # is debited at emission time so the balance tracks temporally.
        eng_busy = {"A": 2600.0, "D": 0.0, "P": 1500.0}
        engs = {}

        def pick(costs, exclude=None):
            elig = {k: v for k, v in costs.items() if k != exclude}
            e = min(elig, key=lambda k: eng_busy[k] + elig[k])
            eng_busy[e] += costs[e]
            return e

        def eng_of(e):
            if not engs:
                engs.update({"A": nc.scalar, "D": nc.vector,
                             "P": nc.gpsimd})
            return engs[e]

        # ---------------- first activation chunk DMAs (critical path) ----
        def x8_src(dram, n, ic):
            # d = s*256 + t*128 + p (fp8 DoubleRow pair layout)
            return bass.AP(
                tensor=dram.tensor,
                offset=dram.offset + n * l + ic * 512,
                ap=[[nlc, 128], [256 * nlc, 4], [128 * nlc, 2], [1, 512]])

        def load_x8(pool_, dram, n, ic):
            t = pool_.tile([128, 4 * 2 * 512], FP8, tag="x8", name="x8t")
            nc.sync.dma_start(
                t.rearrange("p (s t x) -> p s t x", s=4, t=2),
                x8_src(dram, n, ic))
            return t

        wts = {}

        def load_w8(wname, wdram):
            t = wtp.tile([128, 4 * 2 * FC], FP8, tag=wname, name=wname)
            nc.sync.dma_start(
                t.rearrange("p (s t f) -> p s t f", s=4, t=2),
                bass.AP(tensor=wdram.tensor, offset=wdram.offset,
                        ap=[[FC, 128], [256 * FC, 4], [128 * FC, 2],
                            [1, FC]]))
            wts[wname] = t

        load_w8("wq", wq8)
        xq_t = load_x8(xqp, xq8_nd, 0, 0)
        load_w8("wk", wk8)
        xk8_t = load_x8(xk8p, xk8_nd, 0, 0)
        load_w8("wv", wvt)

        # ---------------- constants -------------------------------------
        ones_col = consts.tile([128, 1], BF16)
        nc.vector.memset(ones_col, 1.0)
        eps_sb = consts.tile([128, 1], F32)
        nc.vector.memset(eps_sb, EPS)
        gamma_sb = consts.tile([1, FC], F32)
        nc.sync.dma_start(gamma_sb, gamma)
        beta_sb = consts.tile([1, FC], F32)
        nc.sync.dma_start(beta_sb, beta)
        # lower-triangular (j <= i) mask in [j-part, i-free] layout, fp8
        tm_f = consts.tile([128, 128], F32)
        nc.vector.memset(tm_f, 1.0)
        nc.gpsimd.affine_select(
            out=tm_f, in_=tm_f,
            compare_op=mybir.AluOpType.is_ge, fill=0.0, base=0,
            pattern=[[1, 128]], channel_multiplier=-1)
        trimask = consts.tile([128, 128], FP8)
        nc.vector.tensor_copy(trimask, tm_f)
        # zero operands for the PSUM-bank-clearing matmuls
        zlhs = consts.tile([128, 128], BF16)
        nc.vector.memset(zlhs, 0.0)
        zrhs = consts.tile([128, 512], BF16)
        nc.vector.memset(zrhs, 0.0)
        b8_sb = consts.tile([128, 1], F32)
        nc.vector.memset(b8_sb, B8V)

        def b8bc(*freedims):
            ap = [[b8_sb.ap[0][0], 128]] + [[0, d] for d in freedims]
            return bass.AP(tensor=b8_sb.tensor, offset=b8_sb.offset, ap=ap)

        # ---------------- persistent SBUF -------------------------------
        # kt8_sb: [32p' (x2 heads), (t, n, j)] fp8 DoubleRow pair layout:
        # feature d of head h = h*64 + t*32 + p' lives at partition
        # h*32 + p', column block t.
        kt8_sb = persist.tile([64, 2 * N * l], FP8, tag="kt8")
        # v_sb: [j-in-block, (n, jc, h, 65)] fp8, x8 scaled; col 64 of each
        # 65-group is the baked denominator column (value 8.0)
        v_sb = persist.tile([128, N * jblocks * H2 * 65], FP8, tag="v")
        v3 = v_sb.rearrange("p (g x) -> p g x", x=65)
        nc.gpsimd.memset(v3[:, :, 64:65], VONES)
        # res_sb: [l-in-block, (ls, f)] bf16, ls = n*16 + ic*4 + S
        res_sb = persist.tile([128, nls * FC], BF16, tag="res")
        # qres_sb: the residual input, loaded once: [128, (n, ics, f)]
        qres_sb = persist.tile([128, nls * FC], BF16, tag="qres")

        # statdn bank (2KB, cleared once up front, then start=False only):
        #   cols [0,256): softmax denominator accumulators, one column per
        #                 (ic, n, h, S) - no reuse, so no re-clearing
        #   cols [256,384): per-feature sums (partition 0)
        #   cols [384,512): per-feature sums of squares (partition 0)
        statdn = statp.tile([128, 512], F32, tag="statdn")
        nc.tensor.matmul(statdn, zlhs, zrhs, start=True, stop=True,
                         skip_group_check=True)

        def dn_col(ic, n, h, S):
            return (n * 4 + ic) * 8 + h * 4 + S

        def load_qres(n):
            nc.sync.dma_start(
                qres_sb[:, n * 16 * FC:(n + 1) * 16 * FC],
                bass.AP(tensor=q_res.tensor,
                        offset=q_res.offset + n * 16 * FC,
                        ap=[[nls * FC, 128], [1, 16 * FC]]))

        # ------------------------------------------------------------------
        # projection task machinery (filler micro-ops paced into B loops)
        # ------------------------------------------------------------------
        # projection chains rotate through the same PSUM pool as the score
        # tiles (6 x 2KB bank-slots)
        def alloc_pj():
            return st2p.tile([128, 512], F32, tag="st2", name="pj")

        chain_alloc = {"fn": alloc_pj}

        # fp8 staging tiles for the partition shuffle, per (ic, half)
        qf8_cur = {}
        kf8_cur = {}

        def qk_chain(side, n, ic, get_xt):
            """q/k projection via fp8 DoubleRow, then re-quantize to fp8
            into the shuffle staging tile."""
            w_use = wts["wq"] if side == "q" else wts["wk"]
            w4 = w_use.rearrange("p (s t f) -> p s t f", s=4, t=2)
            pj = {}
            alloc_fn = chain_alloc["fn"]

            def alloc():
                pj["t"] = alloc_fn()

            def mm(s0):
                x4 = get_xt().rearrange("p (s t x) -> p s t x", s=4, t=2)
                for s in (s0, s0 + 1):
                    nc.tensor.matmul(
                        pj["t"], w4[:, s], x4[:, s],
                        start=(s == 0), stop=(s == 3),
                        perf_mode=mybir.MatmulPerfMode.DoubleRow)

            def copy():
                stage = qf8_cur if side == "q" else kf8_cur
                key = (ic, n) if ic == 0 else (ic, n // 2)
                if key not in stage:
                    p_ = qf8p if side == "q" else kf8p
                    nb = 1 if ic == 0 else 2
                    stage[key] = (p_.tile([128, nb * 512], FP8, tag="qf8",
                                          name=f"{side}f8"), nb)
                t, nb = stage[key]
                off = 0 if ic == 0 else (n % 2) * 512
                e = pick(COST_QKCOPY)
                if e == "A":
                    nc.scalar.activation(t[:, off:off + 512], pj["t"],
                                         mybir.ActivationFunctionType.Copy,
                                         scale=SQK)
                else:
                    nc.vector.tensor_scalar_mul(t[:, off:off + 512],
                                                pj["t"], SQK)

            ops = [alloc]
            for s0 in range(0, 4, 2):
                ops.append(lambda s=s0: mm(s))
            ops.append(copy)
            return ops

        def v_chain(n, jsub, ic, get_xt):
            pj = {}
            alloc_fn = chain_alloc["fn"]
            wv4 = wts["wv"].rearrange("p (s t f) -> p s t f", s=4, t=2)

            def alloc():
                pj["t"] = alloc_fn()

            def mm(s0):
                x4 = get_xt().rearrange("p (s t x) -> p s t x", s=4, t=2)
                for s in (s0, s0 + 1):
                    nc.tensor.matmul(
                        pj["t"][:, 0:128],
                        x4[:, s, :, jsub * 128:jsub * 128 + 128],
                        wv4[:, s],
                        start=(s == 0), stop=(s == 3),
                        perf_mode=mybir.MatmulPerfMode.DoubleRow)

            def copy():
                jc = ic * 4 + jsub
                base = (n * jblocks + jc) * H2 * 65
                dst = v_sb[:, base:base + 130].rearrange(
                    "p (h x) -> p h x", h=2)[:, :, 0:64]
                src = pj["t"][:, 0:128].rearrange("p (h x) -> p h x", h=2)
                e = pick(COST_VCOPY)
                if e == "A":
                    nc.scalar.activation(dst, src,
                                         mybir.ActivationFunctionType.Copy,
                                         scale=SV)
                else:
                    nc.vector.tensor_scalar_mul(dst, src, SV)

            ops = [alloc]
            for s0 in range(0, 4, 2):
                ops.append(lambda s=s0: mm(s))
            ops.append(copy)
            return ops

        # qt8 per chunk: [32p' (x2 heads), (t, n, i)] fp8 pair layout
        qt_tiles = {}

        def emit_shuffles(ic, key, batches):
            """DMA the staged q/k fp8 tiles into DoubleRow pair layout.
            The host permutes Wq/Wk columns so projection PSUM partition
            m = t*64 + h*32 + p' holds feature h*64 + t*32 + p'; each t
            half then shuffles with one full-width DMA per side."""
            qt8 = qt_tiles[ic]
            q_stage = qf8_cur.pop(key)[0]
            k_stage = kf8_cur.pop(key)[0]
            nb = len(batches)
            n0 = batches[0]
            for t in range(2):
                src = q_stage[t * 64:(t + 1) * 64]
                dst = qt8[0:64,
                          t * (N * 512) + n0 * 512:
                          t * (N * 512) + n0 * 512 + nb * 512]
                nc.sync.dma_start(dst, src)
            for t in range(2):
                src = k_stage[t * 64:(t + 1) * 64]
                if nb > 1:
                    src = src.rearrange("p (u i) -> p u i", u=nb)
                    dst = bass.AP(
                        tensor=kt8_sb.tensor,
                        offset=kt8_sb.offset + t * (N * l) + n0 * l
                        + ic * 512,
                        ap=[[kt8_sb.ap[0][0], 64], [l, nb], [1, 512]])
                else:
                    dst = kt8_sb[0:64,
                                 t * (N * l) + n0 * l + ic * 512:
                                 t * (N * l) + n0 * l + ic * 512 + 512]
                nc.sync.dma_start(dst, src)

        def build_chunk_groups(ic, first_x):
            """Return per-batch lists of micro-op closures for A(ic)."""
            groups = []
            xq_cur = {0: first_x[0]}
            xk8_cur = {0: first_x[1]}
            for n in range(N):
                chain_alloc["fn"] = alloc_pj
                ops = []
                get_xq = lambda nn=n: xq_cur[nn]
                get_xk8 = lambda nn=n: xk8_cur[nn]
                if n + 1 < N:
                    def pre(nn=n + 1):
                        xq_cur[nn] = load_x8(xqp, xq8_nd, nn, ic)
                        xk8_cur[nn] = load_x8(xk8p, xk8_nd, nn, ic)
                    ops.append(pre)
                if ic == 0:
                    ops.append(lambda nn=n: load_qres(nn))
                ops += qk_chain("q", n, ic, get_xq)
                ops += qk_chain("k", n, ic, get_xk8)
                # shuffle once the staging tile for this group is complete
                if ic == 0:
                    ops.append(lambda nn=n: emit_shuffles(0, (0, nn), [nn]))
                elif n % 2 == 1:
                    ops.append(lambda nn=n: emit_shuffles(
                        ic, (ic, nn // 2), [nn - 1, nn]))
                for jsub in range(4):
                    ops += v_chain(n, jsub, ic, get_xk8)
                groups.append(ops)
            return groups

        # ------------------------------------------------------------------
        # A(0): batch 0's projections + its shuffle run up front; the rest
        # is deadline-paced into B(0).
        # ------------------------------------------------------------------
        qt_tiles[0] = qtp.tile([64, 2 * N * 512], FP8, tag="qt", name="qt")
        groups0 = build_chunk_groups(0, (xq_t, xk8_t))
        for op in groups0[0]:
            op()

        # ------------------------------------------------------------------
        # main loop: one software-pipelined stream over (ic, n, jc).
        # ------------------------------------------------------------------
        specs = []
        for ic in range(ic_n):
            for n in range(N):
                for jc in range(4 * ic + 4):
                    specs.append((ic, n, jc))
        nspec = len(specs)
        st2_of, pair_of, diag_of, avs_of = {}, {}, {}, {}
        sched = []

        def schedule_group(ops, w_start, w_end):
            no = len(ops)
            span = max(1, w_end - w_start)
            for k, op in enumerate(ops):
                sched.append((w_start + (k * span) // no, op))

        def emit_due(idx):
            while sched and sched[0][0] <= idx:
                sched.pop(0)[1]()

        def stage_scores(idx):
            ic, n, jc = specs[idx]
            qt8 = qt_tiles[ic]
            ts = []
            for h in range(H2):
                st2 = st2p.tile([128, 512], F32, tag="st2", name="st2")
                ts.append(st2)
                lhsT = bass.AP(
                    tensor=kt8_sb.tensor,
                    offset=kt8_sb.offset + (h * 32) * kt8_sb.ap[0][0]
                    + n * l + jc * 128,
                    ap=[[kt8_sb.ap[0][0], 32], [N * l, 2], [1, 128]])
                rhs = bass.AP(
                    tensor=qt8.tensor,
                    offset=qt8.offset + (h * 32) * qt8.ap[0][0] + n * 512,
                    ap=[[qt8.ap[0][0], 32], [N * 512, 2], [1, 512]])
                nc.tensor.matmul(
                    st2, lhsT, rhs, start=True, stop=True,
                    perf_mode=mybir.MatmulPerfMode.DoubleRow)
            st2_of[idx] = ts

        def stage_exp(idx):
            ic, n, jc = specs[idx]
            rr = jc - 4 * ic
            sts = st2_of.pop(idx)
            if rr < 0:
                m = jc // 2
                key = (ic, n, m)
                if key not in pair_of:
                    pair_of[key] = at8p.tile([128, 2048], FP8, tag="at8",
                                             name="at8")
                base = (jc % 2) * 1024
                prev = None
                for h in range(H2):
                    dst = pair_of[key][:, base + h * 512:base + h * 512 + 512]
                    eng = pick(COST_FULL_HALF, exclude=prev)
                    prev = eng
                    if eng == "A":
                        nc.scalar.activation(
                            dst, sts[h], mybir.ActivationFunctionType.Exp,
                            scale=EXP_SCALE)
                    else:
                        eng_of(eng).scalar_tensor_tensor(
                            out=dst.bitcast(I8), in0=sts[h], scalar=A8V,
                            in1=b8bc(512),
                            op0=mybir.AluOpType.mult, op1=mybir.AluOpType.add)
            else:
                if rr == 0:
                    diag_of[(ic, n)] = at8dp.tile([128, 4096], FP8,
                                                  tag="at8d", name="at8d")
                t = diag_of[(ic, n)]
                w = (4 - rr) * 128
                prev = None
                for h in range(H2):
                    dst = t[:, rr * 1024 + h * 512 + rr * 128:
                            rr * 1024 + (h + 1) * 512]
                    src = sts[h][:, rr * 128:512]
                    eng = pick(COST_DIAG_HALF[rr], exclude=prev)
                    prev = eng
                    if eng == "A":
                        nc.scalar.activation(
                            dst, src, mybir.ActivationFunctionType.Exp,
                            scale=EXP_SCALE)
                    else:
                        eng_of(eng).scalar_tensor_tensor(
                            out=dst.bitcast(I8), in0=src, scalar=A8V,
                            in1=b8bc(w),
                            op0=mybir.AluOpType.mult, op1=mybir.AluOpType.add)

        def stage_mask(idx):
            # one merged mask multiply per (ic, n): the 4 diagonal blocks'
            # masked squares live at col rr*1152 + h*512 in the shared tile
            ic, n, jc = specs[idx]
            rr = jc - 4 * ic
            if rr != 3:
                return
            t = diag_of[(ic, n)]
            sl = bass.AP(tensor=t.tensor, offset=t.offset,
                         ap=[[t.ap[0][0], 128], [1152, 4], [512, 2],
                             [1, 128]])
            tm = bass.AP(tensor=trimask.tensor, offset=trimask.offset,
                         ap=[[trimask.ap[0][0], 128], [0, 4], [0, 2],
                             [1, 128]])
            eng_of(pick(COST_MASK)).tensor_mul(sl, sl, tm)

        def stage_av(idx):
            ic, n, jc = specs[idx]
            rr = jc - 4 * ic
            if jc == 0:
                avv = avp.tile([128, 512], F32, tag="avv", name="avv")
                avs_of[(ic, n)] = avv
                nc.tensor.matmul(avv, zlhs, zrhs, start=True, stop=True,
                                 skip_group_check=True)
            avv = avs_of[(ic, n)]
            if rr < 0:
                if jc % 2 == 0:
                    return
                pair = pair_of.pop((ic, n, jc // 2))
                vbase = (n * jblocks + (jc - 1)) * H2 * 65
                for h in range(H2):
                    vv = bass.AP(
                        tensor=v_sb.tensor,
                        offset=v_sb.offset + vbase + h * 65,
                        ap=[[v_sb.ap[0][0], 128], [H2 * 65, 2], [1, 64]])
                    vo = bass.AP(
                        tensor=v_sb.tensor,
                        offset=v_sb.offset + vbase + h * 65 + 64,
                        ap=[[v_sb.ap[0][0], 128], [H2 * 65, 2], [1, 1]])
                    for S in range(4):
                        lhsT = bass.AP(
                            tensor=pair.tensor,
                            offset=pair.offset + h * 512 + S * 128,
                            ap=[[pair.ap[0][0], 128], [1024, 2], [1, 128]])
                        nc.tensor.matmul(
                            avv[:, h * 256 + S * 64:h * 256 + S * 64 + 64],
                            lhsT, vv, start=False, stop=False,
                            perf_mode=mybir.MatmulPerfMode.DoubleRow,
                            skip_group_check=True)
                        c = dn_col(ic, n, h, S)
                        nc.tensor.matmul(
                            statdn[:, c:c + 1], lhsT, vo,
                            start=False, stop=False,
                            perf_mode=mybir.MatmulPerfMode.DoubleRow,
                            skip_group_check=True)
            else:
                t = diag_of[(ic, n)] if rr < 3 else diag_of.pop((ic, n))
                vbase = (n * jblocks + jc) * H2 * 65
                for h in range(H2):
                    for S in range(4):
                        if rr > S:
                            continue
                        lhsT = t[:, rr * 1024 + h * 512 + S * 128:
                                 rr * 1024 + h * 512 + S * 128 + 128]
                        nc.tensor.matmul(
                            avv[:, h * 256 + S * 64:h * 256 + S * 64 + 64],
                            lhsT,
                            v_sb[:, vbase + h * 65:vbase + h * 65 + 64],
                            start=False, stop=(rr == S),
                            skip_group_check=True)
                        c = dn_col(ic, n, h, S)
                        nc.tensor.matmul(
                            statdn[:, c:c + 1], lhsT,
                            v_sb[:, vbase + h * 65 + 64:vbase + h * 65 + 65],
                            start=False, stop=(rr == S),
                            skip_group_check=True)
            if jc == 4 * ic + 3:
                enqueue_drain(ic, n)

        # drains and stats run as small deferred pieces, one per iteration
        drain_pending = []

        def enqueue_drain(ic, n):
            avv = avs_of.pop((ic, n))
            base512 = (n * 16 + ic * 4) * FC

            def drain_head(h):
                c0 = dn_col(ic, n, h, 0)
                rec = smallp.tile([128, 4], F32, tag="rec", name="rec")
                nc.vector.reciprocal(rec, statdn[:, c0:c0 + 4])
                eng_busy["D"] += COST_DRAIN["D"]

                def s4(t, off, inner=64):
                    return bass.AP(tensor=t.tensor, offset=t.offset + off,
                                   ap=[[t.ap[0][0], 128], [FC, 4],
                                       [1, inner]])

                # normalized = avv * (1/denom), broadcast per S group
                nt = smallp.tile([128, 256], BF16, tag="nt", name="nt")
                nc.vector.tensor_mul(
                    nt.rearrange("p (s x) -> p s x", s=4),
                    bass.AP(tensor=avv.tensor,
                            offset=avv.offset + h * 256,
                            ap=[[avv.ap[0][0], 128], [64, 4], [1, 64]]),
                    bass.AP(tensor=rec.tensor, offset=rec.offset,
                            ap=[[rec.ap[0][0], 128], [1, 4], [0, 64]]))
                # + residual, strided into res_sb feature slots
                eng_of(pick(COST_DRAIN_ADD)).tensor_add(
                    s4(res_sb, base512 + h * 64),
                    nt.rearrange("p (s x) -> p s x", s=4),
                    s4(qres_sb, base512 + h * 64))

            def drain_sq():
                res_block = res_sb[:, base512:base512 + 512]
                sqt = sqp.tile([128, 512], BF16, tag="sq", name="sqt")
                eng_of(pick(COST_DSQ)).tensor_mul(sqt, res_block, res_block)
                stats_bufs[(ic, n)] = (res_block, sqt)

            def drain_stats():
                res_block, sqt = stats_bufs.pop((ic, n))
                for g in range(4):
                    last = (n == N - 1 and ic == ic_n - 1 and g == 3)
                    nc.tensor.matmul(statdn[0:1, 256:256 + FC], ones_col,
                                     res_block[:, g * FC:(g + 1) * FC],
                                     start=False, stop=last,
                                     skip_group_check=True)
                    nc.tensor.matmul(statdn[0:1, 384:384 + FC], ones_col,
                                     sqt[:, g * FC:(g + 1) * FC],
                                     start=False, stop=last,
                                     skip_group_check=True)

            if DEBUG_NOLAG:
                drain_head(0)
                drain_head(1)
                drain_sq()
                drain_stats()
            else:
                drain_pending.extend(
                    [lambda: drain_head(0), None,
                     lambda: (drain_head(1), drain_sq()), None, drain_stats])

        stats_bufs = {}

        def step_drain(flush=False):
            while drain_pending:
                op = drain_pending.pop(0)
                if op is None:
                    if flush:
                        continue
                    return
                op()

        # spec index of (ic, n, jc=0), for filler deadlines
        start_idx = {}
        for i, (sic, sn, sjc) in enumerate(specs):
            if sjc == 0:
                start_idx[(sic, sn)] = i

        # chunk 0's remaining groups are due just before B(0) reaches
        # batch 2 (their shuffle covers batches 1-3)
        for n in range(1, N):
            schedule_group(groups0[n], start_idx[(0, n - 1)],
                           max(1, start_idx[(0, n)] - 1))

        for idx in range(nspec + max(EXP_LAG, MASK_LAG, AV_LAG)):
            if idx < nspec:
                ic, n, jc = specs[idx]
                if jc == 0 and n == 0 and ic + 1 < ic_n:
                    qt_tiles[ic + 1] = qtp.tile([64, 2 * N * 512], FP8,
                                                tag="qt", name="qt")
                    nxq = load_x8(xqp, xq8_nd, 0, ic + 1)
                    nxk8 = load_x8(xk8p, xk8_nd, 0, ic + 1)
                    groups = build_chunk_groups(ic + 1, (nxq, nxk8))
                    # group g due just before B(ic+1) needs its shuffle:
                    # shuffles fire at the end of groups 1 and 3, needed at
                    # B(ic+1) batches 0 and 2 respectively.
                    w_prev = idx
                    deadlines = [
                        (idx + start_idx[(ic + 1, 0)]) // 2,
                        max(1, start_idx[(ic + 1, 0)] - 1),
                        start_idx[(ic + 1, 1)],
                        max(1, start_idx[(ic + 1, 2)] - 1)]
                    for g in range(N):
                        w_end = deadlines[g]
                        schedule_group(groups[g], w_prev, w_end)
                        w_prev = w_end
                stage_scores(idx)
            if idx - EXP_LAG >= 0 and idx - EXP_LAG < nspec:
                stage_exp(idx - EXP_LAG)
            if idx - MASK_LAG >= 0 and idx - MASK_LAG < nspec:
                stage_mask(idx - MASK_LAG)
            step_drain()
            if idx - AV_LAG >= 0 and idx - AV_LAG < nspec:
                stage_av(idx - AV_LAG)
            emit_due(idx)
        while sched:
            sched.pop(0)[1]()
        step_drain(flush=True)

        # ------------------------------------------------------------------
        # batch-norm: compute gamma', beta', apply
        # ------------------------------------------------------------------
        sumf = statdn[0:1, 256:256 + FC]
        sqf = statdn[0:1, 384:384 + FC]

        inv = 1.0 / NL
        mean = bnp.tile([1, FC], F32, tag="mean", name="mean")
        nc.vector.tensor_scalar_mul(mean, sumf, inv)
        musq = bnp.tile([1, FC], F32, tag="musq", name="musq")
        nc.vector.tensor_mul(musq, mean, mean)
        var = bnp.tile([1, FC], F32, tag="var", name="var")
        nc.vector.scalar_tensor_tensor(
            out=var, in0=sqf, scalar=inv, in1=musq,
            op0=mybir.AluOpType.mult, op1=mybir.AluOpType.subtract)
        std = bnp.tile([1, FC], F32, tag="std", name="std")
        nc.scalar.activation(std, var, mybir.ActivationFunctionType.Sqrt,
                             bias=eps_sb[0:1, :])
        rstd = bnp.tile([1, FC], F32, tag="rstd", name="rstd")
        nc.vector.reciprocal(rstd, std)
        gp = bnp.tile([1, FC], F32, tag="gp", name="gp")
        nc.vector.tensor_mul(gp, gamma_sb, rstd)
        mgp = bnp.tile([1, FC], F32, tag="mgp", name="mgp")
        nc.vector.tensor_mul(mgp, mean, gp)
        bp = bnp.tile([1, FC], F32, tag="bp", name="bp")
        nc.vector.tensor_sub(bp, beta_sb, mgp)
        gp16 = bnp.tile([1, FC], BF16, tag="gp16", name="gp16")
        nc.vector.tensor_copy(gp16, gp)
        bp16 = bnp.tile([1, FC], BF16, tag="bp16", name="bp16")
        nc.vector.tensor_copy(bp16, bp)

        gbc = bnp.tile([128, FC], BF16, tag="gbc", name="gbc")
        nc.gpsimd.partition_broadcast(gbc, gp16)
        bbc = bnp.tile([128, FC], BF16, tag="bbc", name="bbc")
        nc.gpsimd.partition_broadcast(bbc, bp16)

        def rep16(t):
            return bass.AP(tensor=t.tensor, offset=t.offset,
                           ap=[[t.ap[0][0], 128], [0, 16], [1, FC]])

        for n in range(N):
            base = n * 16 * FC
            t1 = outp.tile([128, 16 * FC], BF16, tag="t1", name="t1")
            nc.vector.tensor_mul(t1.rearrange("p (g f) -> p g f", g=16),
                                 res_sb[:, base:base + 16 * FC].rearrange(
                                     "p (g f) -> p g f", g=16),
                                 rep16(gbc))
            ot = outp.tile([128, 16 * FC], BF16, tag="ot", name="ot")
            nc.vector.tensor_add(ot.rearrange("p (g f) -> p g f", g=16), t1.rearrange(
                "p (g f) -> p g f", g=16), rep16(bbc))
            nc.sync.dma_start(
                bass.AP(tensor=out_s.tensor,
                        offset=out_s.offset + n * (16 * FC),
                        ap=[[nls * FC, 128], [1, 16 * FC]]),
                ot)

    nc.compile()
    return nc


def get_runner(nc):
    """Build (once) a cached jitted SPMD executor for the Bass program."""
    if "runner" in _cached:
        return _cached["runner"]

    import jax
    from jax.experimental.shard_map import shard_map
    from jax.sharding import Mesh, PartitionSpec
    from concourse import bass2jax

    bass2jax.install_neuronx_cc_hook()

    partition_name = (nc.partition_id_tensor.name
                      if nc.partition_id_tensor else None)
    in_names, out_names, out_avals, zero_outs = [], [], [], []
    for alloc in nc.m.functions[0].allocations:
        if not isinstance(alloc, mybir.MemoryLocationSet):
            continue
        name = alloc.memorylocations[0].name
        if alloc.kind == "ExternalInput":
            if name != partition_name:
                in_names.append(name)
        elif alloc.kind == "ExternalOutput":
            shape = tuple(alloc.tensor_shape)
            dtype = mybir.dt.np(alloc.dtype)
            out_names.append(name)
            out_avals.append(jax.core.ShapedArray(shape, dtype))
            zero_outs.append(np.zeros(shape, dtype))
    n_params = len(in_names)
    n_outs = len(out_avals)
    all_names = in_names + out_names
    if partition_name is not None:
        all_names = all_names + [partition_name]

    def _body(*args):
        operands = list(args)
        if partition_name is not None:
            operands.append(bass2jax.partition_id_tensor())
        outs = bass2jax._bass_exec_p.bind(
            *operands,
            out_avals=tuple(out_avals),
            in_names=tuple(all_names),
            out_names=tuple(out_names),
            lowering_input_output_aliases=(),
            sim_require_finite=True,
            sim_require_nnan=True,
            nc=nc,
        )
        return tuple(outs)

    devices = jax.devices()[:NCORES]
    mesh = Mesh(np.asarray(devices), ("core",))
    in_specs = (PartitionSpec("core"),) * (n_params + n_outs)
    out_specs = (PartitionSpec("core"),) * n_outs
    donate = tuple(range(n_params, n_params + n_outs))
    sharded = jax.jit(
        shard_map(_body, mesh=mesh, in_specs=in_specs, out_specs=out_specs,
                  check_rep=False),
        donate_argnums=donate, keep_unused=True)

    def run_np(in_maps):
        concat_in = [
            np.concatenate([np.asarray(in_maps[c][nm]) for c in range(NCORES)],
                           axis=0)
            for nm in in_names]
        concat_zeros = [np.zeros((NCORES * z.shape[0], *z.shape[1:]), z.dtype)
                        for z in zero_outs]
        out_arrs = sharded(*concat_in, *concat_zeros)
        return [
            {nm: np.asarray(out_arrs[i]).reshape(
                NCORES, *out_avals[i].shape)[c]
             for i, nm in enumerate(out_names)}
            for c in range(NCORES)]

    _cached["runner"] = (run_np, sharded, in_names, out_names, out_avals,
                         zero_outs, mesh)
    return _cached["runner"]


def make_in_maps(inputs, l):
    query = np.asarray(inputs["query"], dtype=np.float32)
    key = np.asarray(inputs["key"], dtype=np.float32)
    Wq = np.asarray(inputs["Wq"], dtype=np.float32)
    Wk = np.asarray(inputs["Wk"], dtype=np.float32)
    Wv = np.asarray(inputs["Wv"], dtype=np.float32)
    gamma = np.asarray(inputs["gamma"], dtype=np.float32)
    beta = np.asarray(inputs["beta"], dtype=np.float32)

    n = query.shape[0]
    qf = query.reshape(n * l, D)
    kf = key.reshape(n * l, D)
    xq8 = np.ascontiguousarray(qf.T.astype(FP8_NP))
    xk8 = np.ascontiguousarray(kf.T.astype(FP8_NP))

    # Wq/Wk output-column permutation: PSUM partition m = t*64 + h*32 + p'
    # holds feature f = h*64 + t*32 + p' (enables 64-partition shuffle DMAs)
    mm = np.arange(128)
    perm = (((mm % 64) // 32) * 64 + (mm // 64) * 32 + mm % 32)

    in_maps = []
    for c in range(NCORES):
        sl = slice(c * FC, (c + 1) * FC)
        in_maps.append({
            "xq8_nd": xq8,
            "xk8_nd": xk8,
            "wq8": np.ascontiguousarray(
                (Wq[sl][perm].T * W8_SCALE).astype(FP8_NP)),
            "wk8": np.ascontiguousarray(
                (Wk[sl][perm].T * W8_SCALE).astype(FP8_NP)),
            "wvt": np.ascontiguousarray(Wv[sl].T.astype(FP8_NP)),
            # blocked: [p, (n, g, f)] where row = n*l + g*128 + p
            "q_res": np.ascontiguousarray(
                qf[:, sl].astype(BF16_NP).reshape(n, l // 128, 128, FC)
                .transpose(2, 0, 1, 3).reshape(128, -1)),
            "gamma": np.ascontiguousarray(gamma[sl].reshape(1, FC)),
            "beta": np.ascontiguousarray(beta[sl].reshape(1, FC)),
        })
    return in_maps


def kernel(**inputs):
    l = np.asarray(inputs["query"]).shape[1]
    if "nc" not in _cached or _cached.get("l") != l:
        _cached["nc"] = build_program(l)
        _cached["l"] = l
    nc = _cached["nc"]

    in_maps = make_in_maps(inputs, l)
    run_np = get_runner(nc)[0]
    results = run_np(in_maps)

    n = np.asarray(inputs["query"]).shape[0]
    out = np.zeros((n, l, D), dtype=np.float32)
    for c in range(NCORES):
        sl = slice(c * FC, (c + 1) * FC)
        arr = results[c]["out_s"].reshape(128, n, l // 128, FC)
        out[:, :, sl] = arr.transpose(1, 2, 0, 3).reshape(n, l, FC).astype(
            np.float32)
    return out


# revision 81
# speedup vs baseline: 1.0009x; 1.0009x over previous
"""Trainium2 Bass kernel for MultiHeadAttention + residual + BatchNorm.

Model (reference):
  q = query @ Wq.T ; k = key @ Wk.T ; v = key @ Wv.T    (per-head split)
  score = q k^T / sqrt(D), causal mask, softmax over keys
  res   = (attn @ v) + query
  out   = batchnorm(res over all (N*L) rows, per feature) * gamma + beta

Sharding over 8 cores: FEATURE sharding. Core c owns heads {2c, 2c+1}
(features [128c, 128c+128)) for ALL batches. BatchNorm statistics are
then core-local, so no collective is needed at all.

All-fp8 attention datapath: projections use fp8 DoubleRow; projected
q/k are re-quantized to fp8 and DMA-shuffled into a [32-part, 2-pair]
layout (the host permutes Wq/Wk columns so each shuffle is 2 full-width
DMAs) so the score matmuls also run as fp8 DoubleRow (half the PE
columns).  Attention weights are produced in fp8 by ACT (exp writes
fp8 directly) and DVE (e4m3 *bits* via an int8 Schraudolph
scalar_tensor_tensor - valid because |score*scale| is small so the
biased-exponent byte is always in [38, 90]).  Full key blocks are
exp'd pairwise into one [128, 2048] fp8 tile so the attention@V
matmuls run as fp8 DoubleRow over 256 keys at a time.  V carries an
appended constant column whose products accumulate into per-(ic,n,h,S)
columns of a once-cleared PSUM bank, giving softmax denominators
without extra PE passes.

Engine budget: scores/AV/projections on PE; exp split ACT/DVE by a
greedy cost balancer; masks + residual-adds on GPSIMD (which cannot
touch PSUM); DVE carries the PSUM-coupled glue (re-quantize copies,
normalize, reciprocal, batch-norm).  PSUM is exactly packed: 6 score
half-slots (shared with projection accumulators) + attention-value
bank + denominator/stats bank.
"""

import math
import sys

sys.path.insert(0, "/opt/trn_rl_repo")

import numpy as np
import ml_dtypes

import concourse.bass as bass
import concourse.mybir as mybir
from concourse import bacc
import concourse.tile as tile

F32 = mybir.dt.float32
BF16 = mybir.dt.bfloat16
FP8 = mybir.dt.float8e4
I8 = mybir.dt.int8
BF16_NP = ml_dtypes.bfloat16
FP8_NP = mybir.dt.np(FP8)
# q/k weights are scaled by 16 on the host so fp8e4 stays out of the
# subnormal range.
W8_SCALE = 16.0

N = 4
L = 2048
D = 1024
H = 16
P = 64
NCORES = 8
FC = D // NCORES       # features per core = 128
H2 = 2                 # heads per core
EPS = 1e-5
SCALE = 1.0 / math.sqrt(D)
NL = N * L             # 8192 rows in the global batch norm

# fp8 staging scales: q8 = (16 q) * SQK = 8 q ; k8 likewise ; v8 = 8 v.
SQK = 0.5
SV = 8.0
VONES = 8.0            # denominator column value (folded out by 1/(8*sum))
# score PSUM = sum q8 k8 = 64 * raw score -> exp argument scale:
EXP_SCALE = SCALE / 64.0
# e4m3-bit Schraudolph: i8 = round(st2 * A8V + B8V) gives bits of
# ~exp(st2 * EXP_SCALE)
A8V = (8.0 / math.log(2.0)) * EXP_SCALE
B8V = 56.0 - 0.3494 + 0.5

# static per-unit engine-busy costs (ns) for the greedy load balancer.
# NOTE: the Pool/GPSIMD engine cannot access PSUM, so only SBUF-input
# work (masks, the drain residual-add) is Pool-eligible.
COST_FULL_HALF = {"A": 612, "D": 631}
COST_DIAG_HALF = [{"A": 612, "D": 631}, {"A": 505, "D": 504},
                  {"A": 398, "D": 376}, {"A": 292, "D": 247}]
COST_MASK = {"D": 1127, "P": 2126}      # merged, one per (ic, n)
COST_DRAIN = {"D": 523}                 # recip + normalize TT (PSUM: DVE)
COST_DRAIN_ADD = {"D": 127, "P": 603}   # residual add (SBUF)
COST_DSQ = {"D": 193, "P": 1110}        # res^2 for the variance (SBUF)
COST_VCOPY = {"A": 292, "D": 247}
COST_QKCOPY = {"D": 631, "A": 668}

# software-pipeline stage lags
EXP_LAG = 1
MASK_LAG = 4
AV_LAG = 8
DEBUG_NOLAG = False

_cached = {}


def build_program(l=L):
    """Build the SPMD Bass program (identical on all 8 cores)."""
    nc = bacc.Bacc("TRN2", target_bir_lowering=False, debug=False,
                   num_devices=NCORES)

    ic_n = l // 512        # 512-row query chunks
    nlc = N * l            # rows per core (all batches)

    xq8_nd = nc.dram_tensor("xq8_nd", [D, nlc], FP8,
                            kind="ExternalInput").ap()
    xk8_nd = nc.dram_tensor("xk8_nd", [D, nlc], FP8,
                            kind="ExternalInput").ap()
    wq8 = nc.dram_tensor("wq8", [D, FC], FP8, kind="ExternalInput").ap()
    wk8 = nc.dram_tensor("wk8", [D, FC], FP8, kind="ExternalInput").ap()
    wvt = nc.dram_tensor("wvt", [D, FC], FP8, kind="ExternalInput").ap()
    # residual input, pre-blocked on the host to the SBUF res layout:
    # [row-in-block, (n, ic*4+S, f)]
    q_res = nc.dram_tensor("q_res", [128, (nlc // 128) * FC], BF16,
                           kind="ExternalInput").ap()
    gamma = nc.dram_tensor("gamma", [1, FC], F32, kind="ExternalInput").ap()
    beta = nc.dram_tensor("beta", [1, FC], F32, kind="ExternalInput").ap()
    # output keeps the SBUF res layout: [row-in-block, (n, ic, S, f)] so the
    # store is one fat contiguous DMA per batch; the host untangles it.
    out_s = nc.dram_tensor("out_s", [128, (nlc // 128) * FC], BF16,
                           kind="ExternalOutput").ap()

    jblocks = l // 128     # 16 key blocks per batch
    nls = nlc // 128       # 64 ls blocks of res

    from contextlib import ExitStack
    with ExitStack() as stack:
        tc = stack.enter_context(tile.TileContext(nc))
        pool = {}
        for nm, bufs, space in (
                ("consts", 1, None), ("persist", 1, None), ("wt", 1, None),
                ("qtp", 2, None), ("xq", 6, None), ("xk8", 6, None),
                ("qf8", 3, None), ("kf8", 3, None),
                ("at8", 16, None), ("at8d", 4, None),
                ("sq", 4, None),
                ("outp", 2, None), ("bnp", 1, None), ("small", 10, None),
                ("st2", 6, "PSUM"), ("av", 1, "PSUM"),
                ("stat", 1, "PSUM")):
            kw = {"name": nm, "bufs": bufs}
            if space:
                kw["space"] = space
            pool[nm] = stack.enter_context(tc.tile_pool(**kw))
        consts, persist, wtp = pool["consts"], pool["persist"], pool["wt"]
        qtp, xqp, xk8p = pool["qtp"], pool["xq"], pool["xk8"]
        qf8p, kf8p = pool["qf8"], pool["kf8"]
        at8p, at8dp = pool["at8"], pool["at8d"]
        sqp = pool["sq"]
        outp, bnp, smallp = pool["outp"], pool["bnp"], pool["small"]
        st2p, avp, statp = pool["st2"], pool["av"], pool["stat"]

        # greedy engine balancer: assign each elementwise unit to the
        # engine with the least projected busy time.  Hard-assigned work
        # is debited at emission time so the balance tracks temporally.
        eng_busy = {"A": 2600.0, "D": 0.0, "P": 1500.0}
        engs = {}

        def pick(costs, exclude=None):
            elig = {k: v for k, v in costs.items() if k != exclude}
            e = min(elig, key=lambda k: eng_busy[k] + elig[k])
            eng_busy[e] += costs[e]
            return e

        def eng_of(e):
            if not engs:
                engs.update({"A": nc.scalar, "D": nc.vector,
                             "P": nc.gpsimd})
            return engs[e]

        # ---------------- first activation chunk DMAs (critical path) ----
        def x8_src(dram, n, ic):
            # d = s*256 + t*128 + p (fp8 DoubleRow pair layout)
            return bass.AP(
                tensor=dram.tensor,
                offset=dram.offset + n * l + ic * 512,
                ap=[[nlc, 128], [256 * nlc, 4], [128 * nlc, 2], [1, 512]])

        def load_x8(pool_, dram, n, ic):
            t = pool_.tile([128, 4 * 2 * 512], FP8, tag="x8", name="x8t")
            nc.sync.dma_start(
                t.rearrange("p (s t x) -> p s t x", s=4, t=2),
                x8_src(dram, n, ic))
            return t

        wts = {}

        def load_w8(wname, wdram):
            t = wtp.tile([128, 4 * 2 * FC], FP8, tag=wname, name=wname)
            nc.sync.dma_start(
                t.rearrange("p (s t f) -> p s t f", s=4, t=2),
                bass.AP(tensor=wdram.tensor, offset=wdram.offset,
                        ap=[[FC, 128], [256 * FC, 4], [128 * FC, 2],
                            [1, FC]]))
            wts[wname] = t

        load_w8("wq", wq8)
        xq_t = load_x8(xqp, xq8_nd, 0, 0)
        load_w8("wk", wk8)
        xk8_t = load_x8(xk8p, xk8_nd, 0, 0)
        load_w8("wv", wvt)

        # ---------------- constants -------------------------------------
        ones_col = consts.tile([128, 1], BF16)
        nc.vector.memset(ones_col, 1.0)
        eps_sb = consts.tile([128, 1], F32)
        nc.vector.memset(eps_sb, EPS)
        gamma_sb = consts.tile([1, FC], F32)
        nc.sync.dma_start(gamma_sb, gamma)
        beta_sb = consts.tile([1, FC], F32)
        nc.sync.dma_start(beta_sb, beta)
        # lower-triangular (j <= i) mask in [j-part, i-free] layout, fp8
        tm_f = consts.tile([128, 128], F32)
        nc.vector.memset(tm_f, 1.0)
        nc.gpsimd.affine_select(
            out=tm_f, in_=tm_f,
            compare_op=mybir.AluOpType.is_ge, fill=0.0, base=0,
            pattern=[[1, 128]], channel_multiplier=-1)
        trimask = consts.tile([128, 128], FP8)
        nc.vector.tensor_copy(trimask, tm_f)
        # zero operands for the PSUM-bank-clearing matmuls
        zlhs = consts.tile([128, 128], BF16)
        nc.vector.memset(zlhs, 0.0)
        zrhs = consts.tile([128, 512], BF16)
        nc.vector.memset(zrhs, 0.0)
        b8_sb = consts.tile([128, 1], F32)
        nc.vector.memset(b8_sb, B8V)

        def b8bc(*freedims):
            ap = [[b8_sb.ap[0][0], 128]] + [[0, d] for d in freedims]
            return bass.AP(tensor=b8_sb.tensor, offset=b8_sb.offset, ap=ap)

        # ---------------- persistent SBUF -------------------------------
        # kt8_sb: [32p' (x2 heads), (t, n, j)] fp8 DoubleRow pair layout:
        # feature d of head h = h*64 + t*32 + p' lives at partition
        # h*32 + p', column block t.
        kt8_sb = persist.tile([64, 2 * N * l], FP8, tag="kt8")
        # v_sb: [j-in-block, (n, jc, h, 65)] fp8, x8 scaled; col 64 of each
        # 65-group is the baked denominator column (value 8.0)
        v_sb = persist.tile([128, N * jblocks * H2 * 65], FP8, tag="v")
        v3 = v_sb.rearrange("p (g x) -> p g x", x=65)
        nc.gpsimd.memset(v3[:, :, 64:65], VONES)
        # res_sb: [l-in-block, (ls, f)] bf16, ls = n*16 + ic*4 + S
        res_sb = persist.tile([128, nls * FC], BF16, tag="res")
        # qres_sb: the residual input, loaded once: [128, (n, ics, f)]
        qres_sb = persist.tile([128, nls * FC], BF16, tag="qres")

        # statdn bank (2KB, cleared once up front, then start=False only):
        #   cols [0,256): softmax denominator accumulators, one column per
        #                 (ic, n, h, S) - no reuse, so no re-clearing
        #   cols [256,384): per-feature sums (partition 0)
        #   cols [384,512): per-feature sums of squares (partition 0)
        statdn = statp.tile([128, 512], F32, tag="statdn")
        nc.tensor.matmul(statdn, zlhs, zrhs, start=True, stop=True,
                         skip_group_check=True)

        def dn_col(ic, n, h, S):
            return (n * 4 + ic) * 8 + h * 4 + S

        def load_qres(n):
            nc.sync.dma_start(
                qres_sb[:, n * 16 * FC:(n + 1) * 16 * FC],
                bass.AP(tensor=q_res.tensor,
                        offset=q_res.offset + n * 16 * FC,
                        ap=[[nls * FC, 128], [1, 16 * FC]]))

        # ------------------------------------------------------------------
        # projection task machinery (filler micro-ops paced into B loops)
        # ------------------------------------------------------------------
        # projection chains rotate through the same PSUM pool as the score
        # tiles (6 x 2KB bank-slots)
        def alloc_pj():
            return st2p.tile([128, 512], F32, tag="st2", name="pj")

        chain_alloc = {"fn": alloc_pj}

        # fp8 staging tiles for the partition shuffle, per (ic, half)
        qf8_cur = {}
        kf8_cur = {}

        def qk_chain(side, n, ic, get_xt):
            """q/k projection via fp8 DoubleRow, then re-quantize to fp8
            into the shuffle staging tile."""
            w_use = wts["wq"] if side == "q" else wts["wk"]
            w4 = w_use.rearrange("p (s t f) -> p s t f", s=4, t=2)
            pj = {}
            alloc_fn = chain_alloc["fn"]

            def alloc():
                pj["t"] = alloc_fn()

            def mm(s0):
                x4 = get_xt().rearrange("p (s t x) -> p s t x", s=4, t=2)
                for s in (s0, s0 + 1):
                    nc.tensor.matmul(
                        pj["t"], w4[:, s], x4[:, s],
                        start=(s == 0), stop=(s == 3),
                        perf_mode=mybir.MatmulPerfMode.DoubleRow)

            def copy():
                stage = qf8_cur if side == "q" else kf8_cur
                key = (ic, n) if ic == 0 else (ic, n // 2)
                if key not in stage:
                    p_ = qf8p if side == "q" else kf8p
                    nb = 1 if ic == 0 else 2
                    stage[key] = (p_.tile([128, nb * 512], FP8, tag="qf8",
                                          name=f"{side}f8"), nb)
                t, nb = stage[key]
                off = 0 if ic == 0 else (n % 2) * 512
                e = pick(COST_QKCOPY)
                if e == "A":
                    nc.scalar.activation(t[:, off:off + 512], pj["t"],
                                         mybir.ActivationFunctionType.Copy,
                                         scale=SQK)
                else:
                    nc.vector.tensor_scalar_mul(t[:, off:off + 512],
                                                pj["t"], SQK)

            ops = [alloc]
            for s0 in range(0, 4, 2):
                ops.append(lambda s=s0: mm(s))
            ops.append(copy)
            return ops

        def v_chain(n, jsub, ic, get_xt):
            pj = {}
            alloc_fn = chain_alloc["fn"]
            wv4 = wts["wv"].rearrange("p (s t f) -> p s t f", s=4, t=2)

            def alloc():
                pj["t"] = alloc_fn()

            def mm(s0):
                x4 = get_xt().rearrange("p (s t x) -> p s t x", s=4, t=2)
                for s in (s0, s0 + 1):
                    nc.tensor.matmul(
                        pj["t"][:, 0:128],
                        x4[:, s, :, jsub * 128:jsub * 128 + 128],
                        wv4[:, s],
                        start=(s == 0), stop=(s == 3),
                        perf_mode=mybir.MatmulPerfMode.DoubleRow)

            def copy():
                jc = ic * 4 + jsub
                base = (n * jblocks + jc) * H2 * 65
                dst = v_sb[:, base:base + 130].rearrange(
                    "p (h x) -> p h x", h=2)[:, :, 0:64]
                src = pj["t"][:, 0:128].rearrange("p (h x) -> p h x", h=2)
                e = pick(COST_VCOPY)
                if e == "A":
                    nc.scalar.activation(dst, src,
                                         mybir.ActivationFunctionType.Copy,
                                         scale=SV)
                else:
                    nc.vector.tensor_scalar_mul(dst, src, SV)

            ops = [alloc]
            for s0 in range(0, 4, 2):
                ops.append(lambda s=s0: mm(s))
            ops.append(copy)
            return ops

        # qt8 per chunk: [32p' (x2 heads), (t, n, i)] fp8 pair layout
        qt_tiles = {}

        def emit_shuffles(ic, key, batches):
            """DMA the staged q/k fp8 tiles into DoubleRow pair layout.
            The host permutes Wq/Wk columns so projection PSUM partition
            m = t*64 + h*32 + p' holds feature h*64 + t*32 + p'; each t
            half then shuffles with one full-width DMA per side."""
            qt8 = qt_tiles[ic]
            q_stage = qf8_cur.pop(key)[0]
            k_stage = kf8_cur.pop(key)[0]
            nb = len(batches)
            n0 = batches[0]
            for t in range(2):
                src = q_stage[t * 64:(t + 1) * 64]
                dst = qt8[0:64,
                          t * (N * 512) + n0 * 512:
                          t * (N * 512) + n0 * 512 + nb * 512]
                nc.sync.dma_start(dst, src)
            for t in range(2):
                src = k_stage[t * 64:(t + 1) * 64]
                if nb > 1:
                    src = src.rearrange("p (u i) -> p u i", u=nb)
                    dst = bass.AP(
                        tensor=kt8_sb.tensor,
                        offset=kt8_sb.offset + t * (N * l) + n0 * l
                        + ic * 512,
                        ap=[[kt8_sb.ap[0][0], 64], [l, nb], [1, 512]])
                else:
                    dst = kt8_sb[0:64,
                                 t * (N * l) + n0 * l + ic * 512:
                                 t * (N * l) + n0 * l + ic * 512 + 512]
                nc.sync.dma_start(dst, src)

        def build_chunk_groups(ic, first_x):
            """Return per-batch lists of micro-op closures for A(ic)."""
            groups = []
            xq_cur = {0: first_x[0]}
            xk8_cur = {0: first_x[1]}
            for n in range(N):
                chain_alloc["fn"] = alloc_pj
                ops = []
                get_xq = lambda nn=n: xq_cur[nn]
                get_xk8 = lambda nn=n: xk8_cur[nn]
                if n + 1 < N:
                    def pre(nn=n + 1):
                        xq_cur[nn] = load_x8(xqp, xq8_nd, nn, ic)
                        xk8_cur[nn] = load_x8(xk8p, xk8_nd, nn, ic)
                    ops.append(pre)
                if ic == 0:
                    ops.append(lambda nn=n: load_qres(nn))
                ops += qk_chain("q", n, ic, get_xq)
                ops += qk_chain("k", n, ic, get_xk8)
                # shuffle once the staging tile for this group is complete
                if ic == 0:
                    ops.append(lambda nn=n: emit_shuffles(0, (0, nn), [nn]))
                elif n % 2 == 1:
                    ops.append(lambda nn=n: emit_shuffles(
                        ic, (ic, nn // 2), [nn - 1, nn]))
                for jsub in range(4):
                    ops += v_chain(n, jsub, ic, get_xk8)
                groups.append(ops)
            return groups

        # ------------------------------------------------------------------
        # A(0): batch 0's projections + its shuffle run up front; the rest
        # is deadline-paced into B(0).
        # ------------------------------------------------------------------
        qt_tiles[0] = qtp.tile([64, 2 * N * 512], FP8, tag="qt", name="qt")
        groups0 = build_chunk_groups(0, (xq_t, xk8_t))
        for op in groups0[0]:
            op()

        # ------------------------------------------------------------------
        # main loop: one software-pipelined stream over (ic, n, jc).
        # ------------------------------------------------------------------
        specs = []
        for ic in range(ic_n):
            for n in range(N):
                for jc in range(4 * ic + 4):
                    specs.append((ic, n, jc))
        nspec = len(specs)
        st2_of, pair_of, diag_of, avs_of = {}, {}, {}, {}
        sched = []

        def schedule_group(ops, w_start, w_end):
            no = len(ops)
            span = max(1, w_end - w_start)
            for k, op in enumerate(ops):
                sched.append((w_start + (k * span) // no, op))

        def emit_due(idx):
            while sched and sched[0][0] <= idx:
                sched.pop(0)[1]()

        def stage_scores(idx):
            ic, n, jc = specs[idx]
            qt8 = qt_tiles[ic]
            ts = []
            for h in range(H2):
                st2 = st2p.tile([128, 512], F32, tag="st2", name="st2")
                ts.append(st2)
                lhsT = bass.AP(
                    tensor=kt8_sb.tensor,
                    offset=kt8_sb.offset + (h * 32) * kt8_sb.ap[0][0]
                    + n * l + jc * 128,
                    ap=[[kt8_sb.ap[0][0], 32], [N * l, 2], [1, 128]])
                rhs = bass.AP(
                    tensor=qt8.tensor,
                    offset=qt8.offset + (h * 32) * qt8.ap[0][0] + n * 512,
                    ap=[[qt8.ap[0][0], 32], [N * 512, 2], [1, 512]])
                nc.tensor.matmul(
                    st2, lhsT, rhs, start=True, stop=True,
                    perf_mode=mybir.MatmulPerfMode.DoubleRow)
            st2_of[idx] = ts

        def stage_exp(idx):
            ic, n, jc = specs[idx]
            rr = jc - 4 * ic
            sts = st2_of.pop(idx)
            if rr < 0:
                m = jc // 2
                key = (ic, n, m)
                if key not in pair_of:
                    pair_of[key] = at8p.tile([128, 2048], FP8, tag="at8",
                                             name="at8")
                base = (jc % 2) * 1024
                prev = None
                for h in range(H2):
                    dst = pair_of[key][:, base + h * 512:base + h * 512 + 512]
                    eng = pick(COST_FULL_HALF, exclude=prev)
                    prev = eng
                    if eng == "A":
                        nc.scalar.activation(
                            dst, sts[h], mybir.ActivationFunctionType.Exp,
                            scale=EXP_SCALE)
                    else:
                        eng_of(eng).scalar_tensor_tensor(
                            out=dst.bitcast(I8), in0=sts[h], scalar=A8V,
                            in1=b8bc(512),
                            op0=mybir.AluOpType.mult, op1=mybir.AluOpType.add)
            else:
                if rr == 0:
                    diag_of[(ic, n)] = at8dp.tile([128, 4096], FP8,
                                                  tag="at8d", name="at8d")
                t = diag_of[(ic, n)]
                w = (4 - rr) * 128
                prev = None
                for h in range(H2):
                    dst = t[:, rr * 1024 + h * 512 + rr * 128:
                            rr * 1024 + (h + 1) * 512]
                    src = sts[h][:, rr * 128:512]
                    eng = pick(COST_DIAG_HALF[rr], exclude=prev)
                    prev = eng
                    if eng == "A":
                        nc.scalar.activation(
                            dst, src, mybir.ActivationFunctionType.Exp,
                            scale=EXP_SCALE)
                    else:
                        eng_of(eng).scalar_tensor_tensor(
                            out=dst.bitcast(I8), in0=src, scalar=A8V,
                            in1=b8bc(w),
                            op0=mybir.AluOpType.mult, op1=mybir.AluOpType.add)

        def stage_mask(idx):
            # one merged mask multiply per (ic, n): the 4 diagonal blocks'
            # masked squares live at col rr*1152 + h*512 in the shared tile
            ic, n, jc = specs[idx]
            rr = jc - 4 * ic
            if rr != 3:
                return
            t = diag_of[(ic, n)]
            sl = bass.AP(tensor=t.tensor, offset=t.offset,
                         ap=[[t.ap[0][0], 128], [1152, 4], [512, 2],
                             [1, 128]])
            tm = bass.AP(tensor=trimask.tensor, offset=trimask.offset,
                         ap=[[trimask.ap[0][0], 128], [0, 4], [0, 2],
                             [1, 128]])
            eng_of(pick(COST_MASK)).tensor_mul(sl, sl, tm)

        def stage_av(idx):
            ic, n, jc = specs[idx]
            rr = jc - 4 * ic
            if jc == 0:
                avv = avp.tile([128, 512], F32, tag="avv", name="avv")
                avs_of[(ic, n)] = avv
                nc.tensor.matmul(avv, zlhs, zrhs, start=True, stop=True,
                                 skip_group_check=True)
            avv = avs_of[(ic, n)]
            if rr < 0:
                if jc % 2 == 0:
                    return
                pair = pair_of.pop((ic, n, jc // 2))
                vbase = (n * jblocks + (jc - 1)) * H2 * 65
                for h in range(H2):
                    vv = bass.AP(
                        tensor=v_sb.tensor,
                        offset=v_sb.offset + vbase + h * 65,
                        ap=[[v_sb.ap[0][0], 128], [H2 * 65, 2], [1, 64]])
                    vo = bass.AP(
                        tensor=v_sb.tensor,
                        offset=v_sb.offset + vbase + h * 65 + 64,
                        ap=[[v_sb.ap[0][0], 128], [H2 * 65, 2], [1, 1]])
                    for S in range(4):
                        lhsT = bass.AP(
                            tensor=pair.tensor,
                            offset=pair.offset + h * 512 + S * 128,
                            ap=[[pair.ap[0][0], 128], [1024, 2], [1, 128]])
                        nc.tensor.matmul(
                            avv[:, h * 256 + S * 64:h * 256 + S * 64 + 64],
                            lhsT, vv, start=False, stop=False,
                            perf_mode=mybir.MatmulPerfMode.DoubleRow,
                            skip_group_check=True)
                        c = dn_col(ic, n, h, S)
                        nc.tensor.matmul(
                            statdn[:, c:c + 1], lhsT, vo,
                            start=False, stop=False,
                            perf_mode=mybir.MatmulPerfMode.DoubleRow,
                            skip_group_check=True)
            else:
                t = diag_of[(ic, n)] if rr < 3 else diag_of.pop((ic, n))
                vbase = (n * jblocks + jc) * H2 * 65
                for h in range(H2):
                    for S in range(4):
                        if rr > S:
                            continue
                        lhsT = t[:, rr * 1024 + h * 512 + S * 128:
                                 rr * 1024 + h * 512 + S * 128 + 128]
                        nc.tensor.matmul(
                            avv[:, h * 256 + S * 64:h * 256 + S * 64 + 64],
                            lhsT,
                            v_sb[:, vbase + h * 65:vbase + h * 65 + 64],
                            start=False, stop=(rr == S),
                            skip_group_check=True)
                        c = dn_col(ic, n, h, S)
                        nc.tensor.matmul(
                            statdn[:, c:c + 1], lhsT,
                            v_sb[:, vbase + h * 65 + 64:vbase + h * 65 + 65],
                            start=False, stop=(rr == S),
                            skip_group_check=True)
            if jc == 4 * ic + 3:
                enqueue_drain(ic, n)

        # drains and stats run as small deferred pieces, one per iteration
        drain_pending = []

        def enqueue_drain(ic, n):
            avv = avs_of.pop((ic, n))
            base512 = (n * 16 + ic * 4) * FC

            def drain_head(h):
                c0 = dn_col(ic, n, h, 0)
                rec = smallp.tile([128, 4], F32, tag="rec", name="rec")
                nc.vector.reciprocal(rec, statdn[:, c0:c0 + 4])
                eng_busy["D"] += COST_DRAIN["D"]

                def s4(t, off, inner=64):
                    return bass.AP(tensor=t.tensor, offset=t.offset + off,
                                   ap=[[t.ap[0][0], 128], [FC, 4],
                                       [1, inner]])

                # normalized = avv * (1/denom), broadcast per S group
                nt = smallp.tile([128, 256], BF16, tag="nt", name="nt")
                nc.vector.tensor_mul(
                    nt.rearrange("p (s x) -> p s x", s=4),
                    bass.AP(tensor=avv.tensor,
                            offset=avv.offset + h * 256,
                            ap=[[avv.ap[0][0], 128], [64, 4], [1, 64]]),
                    bass.AP(tensor=rec.tensor, offset=rec.offset,
                            ap=[[rec.ap[0][0], 128], [1, 4], [0, 64]]))
                # + residual, strided into res_sb feature slots
                eng_of(pick(COST_DRAIN_ADD)).tensor_add(
                    s4(res_sb, base512 + h * 64),
                    nt.rearrange("p (s x) -> p s x", s=4),
                    s4(qres_sb, base512 + h * 64))

            def drain_sq():
                res_block = res_sb[:, base512:base512 + 512]
                sqt = sqp.tile([128, 512], BF16, tag="sq", name="sqt")
                eng_of(pick(COST_DSQ)).tensor_mul(sqt, res_block, res_block)
                stats_bufs[(ic, n)] = (res_block, sqt)

            def drain_stats():
                res_block, sqt = stats_bufs.pop((ic, n))
                for g in range(4):
                    last = (n == N - 1 and ic == ic_n - 1 and g == 3)
                    nc.tensor.matmul(statdn[0:1, 256:256 + FC], ones_col,
                                     res_block[:, g * FC:(g + 1) * FC],
                                     start=False, stop=last,
                                     skip_group_check=True)
                    nc.tensor.matmul(statdn[0:1, 384:384 + FC], ones_col,
                                     sqt[:, g * FC:(g + 1) * FC],
                                     start=False, stop=last,
                                     skip_group_check=True)

            if DEBUG_NOLAG:
                drain_head(0)
                drain_head(1)
                drain_sq()
                drain_stats()
            else:
                drain_pending.extend(
                    [lambda: drain_head(0), None,
                     lambda: (drain_head(1), drain_sq()), None, drain_stats])

        stats_bufs = {}

        def step_drain(flush=False):
            while drain_pending:
                op = drain_pending.pop(0)
                if op is None:
                    if flush:
                        continue
                    return
                op()

        # spec index of (ic, n, jc=0), for filler deadlines
        start_idx = {}
        for i, (sic, sn, sjc) in enumerate(specs):
            if sjc == 0:
                start_idx[(sic, sn)] = i

        # chunk 0's remaining groups are due just before B(0) reaches
        # batch 2 (their shuffle covers batches 1-3)
        for n in range(1, N):
            schedule_group(groups0[n], start_idx[(0, n - 1)],
                           max(1, start_idx[(0, n)] - 1))

        for idx in range(nspec + max(EXP_LAG, MASK_LAG, AV_LAG)):
            if idx < nspec:
                ic, n, jc = specs[idx]
                if jc == 0 and n == 0 and ic + 1 < ic_n:
                    qt_tiles[ic + 1] = qtp.tile([64, 2 * N * 512], FP8,
                                                tag="qt", name="qt")
                    nxq = load_x8(xqp, xq8_nd, 0, ic + 1)
                    nxk8 = load_x8(xk8p, xk8_nd, 0, ic + 1)
                    groups = build_chunk_groups(ic + 1, (nxq, nxk8))
                    # group g due just before B(ic+1) needs its shuffle:
                    # shuffles fire at the end of groups 1 and 3, needed at
                    # B(ic+1) batches 0 and 2 respectively.
                    w_prev = idx
                    deadlines = [
                        (idx + start_idx[(ic + 1, 0)]) // 2,
                        max(1, start_idx[(ic + 1, 0)] - 1),
                        start_idx[(ic + 1, 1)],
                        max(1, start_idx[(ic + 1, 2)] - 1)]
                    for g in range(N):
                        w_end = deadlines[g]
                        schedule_group(groups[g], w_prev, w_end)
                        w_prev = w_end
                stage_scores(idx)
            if idx - EXP_LAG >= 0 and idx - EXP_LAG < nspec:
                stage_exp(idx - EXP_LAG)
            if idx - MASK_LAG >= 0 and idx - MASK_LAG < nspec:
                stage_mask(idx - MASK_LAG)
            step_drain()
            if idx - AV_LAG >= 0 and idx - AV_LAG < nspec:
                stage_av(idx - AV_LAG)
            emit_due(idx)
        while sched:
            sched.pop(0)[1]()
        step_drain(flush=True)

        # ------------------------------------------------------------------
        # batch-norm: compute gamma', beta', apply
        # ------------------------------------------------------------------
        sumf = statdn[0:1, 256:256 + FC]
        sqf = statdn[0:1, 384:384 + FC]

        inv = 1.0 / NL
        mean = bnp.tile([1, FC], F32, tag="mean", name="mean")
        nc.vector.tensor_scalar_mul(mean, sumf, inv)
        musq = bnp.tile([1, FC], F32, tag="musq", name="musq")
        nc.vector.tensor_mul(musq, mean, mean)
        var = bnp.tile([1, FC], F32, tag="var", name="var")
        nc.vector.scalar_tensor_tensor(
            out=var, in0=sqf, scalar=inv, in1=musq,
            op0=mybir.AluOpType.mult, op1=mybir.AluOpType.subtract)
        std = bnp.tile([1, FC], F32, tag="std", name="std")
        nc.scalar.activation(std, var, mybir.ActivationFunctionType.Sqrt,
                             bias=eps_sb[0:1, :])
        rstd = bnp.tile([1, FC], F32, tag="rstd", name="rstd")
        nc.vector.reciprocal(rstd, std)
        gp = bnp.tile([1, FC], F32, tag="gp", name="gp")
        nc.vector.tensor_mul(gp, gamma_sb, rstd)
        mgp = bnp.tile([1, FC], F32, tag="mgp", name="mgp")
        nc.vector.tensor_mul(mgp, mean, gp)
        bp = bnp.tile([1, FC], F32, tag="bp", name="bp")
        nc.vector.tensor_sub(bp, beta_sb, mgp)
        gp16 = bnp.tile([1, FC], BF16, tag="gp16", name="gp16")
        nc.vector.tensor_copy(gp16, gp)
        bp16 = bnp.tile([1, FC], BF16, tag="bp16", name="bp16")
        nc.vector.tensor_copy(bp16, bp)

        gbc = bnp.tile([128, FC], BF16, tag="gbc", name="gbc")
        nc.gpsimd.partition_broadcast(gbc, gp16)
        bbc = bnp.tile([128, FC], BF16, tag="bbc", name="bbc")
        nc.gpsimd.partition_broadcast(bbc, bp16)

        def rep16(t):
            return bass.AP(tensor=t.tensor, offset=t.offset,
                           ap=[[t.ap[0][0], 128], [0, 16], [1, FC]])

        for n in range(N):
            base = n * 16 * FC
            t1 = outp.tile([128, 16 * FC], BF16, tag="t1", name="t1")
            nc.vector.tensor_mul(t1.rearrange("p (g f) -> p g f", g=16),
                                 res_sb[:, base:base + 16 * FC].rearrange(
                                     "p (g f) -> p g f", g=16),
                                 rep16(gbc))
            ot = outp.tile([128, 16 * FC], BF16, tag="ot", name="ot")
            nc.vector.tensor_add(ot.rearrange("p (g f) -> p g f", g=16), t1.rearrange(
                "p (g f) -> p g f", g=16), rep16(bbc))
            nc.sync.dma_start(
                bass.AP(tensor=out_s.tensor,
                        offset=out_s.offset + n * (16 * FC),
                        ap=[[nls * FC, 128], [1, 16 * FC]]),
                ot)

    nc.compile()
    return nc


def get_runner(nc):
    """Build (once) a cached jitted SPMD executor for the Bass program."""
    if "runner" in _cached:
        return _cached["runner"]

    import jax
    from jax.experimental.shard_map import shard_map
    from jax.sharding import Mesh, PartitionSpec
    from concourse import bass2jax

    bass2jax.install_neuronx_cc_hook()

    partition_name = (nc.partition_id_tensor.name
                      if nc.partition_id_tensor else None)
    in_names, out_names, out_avals, zero_outs = [], [], [], []
    for alloc in nc.m.functions[0].allocations:
        if not isinstance(alloc, mybir.MemoryLocationSet):
            continue
        name = alloc.memorylocations[0].name
        if alloc.kind == "ExternalInput":
            if name != partition_name:
                in_names.append(name)
        elif alloc.kind == "ExternalOutput":
            shape = tuple(alloc.tensor_shape)
            dtype = mybir.dt.np(alloc.dtype)
            out_names.append(name)
            out_avals.append(jax.core.ShapedArray(shape, dtype))
            zero_outs.append(np.zeros(shape, dtype))
    n_params = len(in_names)
    n_outs = len(out_avals)
    all_names = in_names + out_names
    if partition_name is not None:
        all_names = all_names + [partition_name]

    def _body(*args):
        operands = list(args)
        if partition_name is not None:
            operands.append(bass2jax.partition_id_tensor())
        outs = bass2jax._bass_exec_p.bind(
            *operands,
            out_avals=tuple(out_avals),
            in_names=tuple(all_names),
            out_names=tuple(out_names),
            lowering_input_output_aliases=(),
            sim_require_finite=True,
            sim_require_nnan=True,
            nc=nc,
        )
        return tuple(outs)

    devices = jax.devices()[:NCORES]
    mesh = Mesh(np.asarray(devices), ("core",))
    in_specs = (PartitionSpec("core"),) * (n_params + n_outs)
    out_specs = (PartitionSpec("core"),) * n_outs
    donate = tuple(range(n_params, n_params + n_outs))
    sharded = jax.jit(
        shard_map(_body, mesh=mesh, in_specs=in_specs, out_specs=out_specs,
                  check_rep=False),
        donate_argnums=donate, keep_unused=True)

    def run_np(in_maps):
        concat_in = [
            np.concatenate([np.asarray(in_maps[c][nm]) for c in range(NCORES)],
                           axis=0)
            for nm in in_names]
        concat_zeros = [np.zeros((NCORES * z.shape[0], *z.shape[1:]), z.dtype)
                        for z in zero_outs]
        out_arrs = sharded(*concat_in, *concat_zeros)
        return [
            {nm: np.asarray(out_arrs[i]).reshape(
                NCORES, *out_avals[i].shape)[c]
             for i, nm in enumerate(out_names)}
            for c in range(NCORES)]

    _cached["runner"] = (run_np, sharded, in_names, out_names, out_avals,
                         zero_outs, mesh)
    return _cached["runner"]


def make_in_maps(inputs, l):
    query = np.asarray(inputs["query"], dtype=np.float32)
    key = np.asarray(inputs["key"], dtype=np.float32)
    Wq = np.asarray(inputs["Wq"], dtype=np.float32)
    Wk = np.asarray(inputs["Wk"], dtype=np.float32)
    Wv = np.asarray(inputs["Wv"], dtype=np.float32)
    gamma = np.asarray(inputs["gamma"], dtype=np.float32)
    beta = np.asarray(inputs["beta"], dtype=np.float32)

    n = query.shape[0]
    qf = query.reshape(n * l, D)
    kf = key.reshape(n * l, D)
    xq8 = np.ascontiguousarray(qf.T.astype(FP8_NP))
    xk8 = np.ascontiguousarray(kf.T.astype(FP8_NP))

    # Wq/Wk output-column permutation: PSUM partition m = t*64 + h*32 + p'
    # holds feature f = h*64 + t*32 + p' (enables 64-partition shuffle DMAs)
    mm = np.arange(128)
    perm = (((mm % 64) // 32) * 64 + (mm // 64) * 32 + mm % 32)

    in_maps = []
    for c in range(NCORES):
        sl = slice(c * FC, (c + 1) * FC)
        in_maps.append({
            "xq8_nd": xq8,
            "xk8_nd": xk8,
            "wq8": np.ascontiguousarray(
                (Wq[sl][perm].T * W8_SCALE).astype(FP8_NP)),
            "wk8": np.ascontiguousarray(
                (Wk[sl][perm].T * W8_SCALE).astype(FP8_NP)),
            "wvt": np.ascontiguousarray(Wv[sl].T.astype(FP8_NP)),
            # blocked: [p, (n, g, f)] where row = n*l + g*128 + p
            "q_res": np.ascontiguousarray(
                qf[:, sl].astype(BF16_NP).reshape(n, l // 128, 128, FC)
                .transpose(2, 0, 1, 3).reshape(128, -1)),
            "gamma": np.ascontiguousarray(gamma[sl].reshape(1, FC)),
            "beta": np.ascontiguousarray(beta[sl].reshape(1, FC)),
        })
    return in_maps


def kernel(**inputs):
    l = np.asarray(inputs["query"]).shape[1]
    if "nc" not in _cached or _cached.get("l") != l:
        _cached["nc"] = build_program(l)
        _cached["l"] = l
    nc = _cached["nc"]

    in_maps = make_in_maps(inputs, l)
    run_np = get_runner(nc)[0]
    results = run_np(in_maps)

    n = np.asarray(inputs["query"]).shape[0]
    out = np.zeros((n, l, D), dtype=np.float32)
    for c in range(NCORES):
        sl = slice(c * FC, (c + 1) * FC)
        arr = results[c]["out_s"].reshape(128, n, l // 128, FC)
        out[:, :, sl] = arr.transpose(1, 2, 0, 3).reshape(n, l, FC).astype(
            np.float32)
    return out


# revision 84
# speedup vs baseline: 1.0067x; 1.0058x over previous
"""Trainium2 Bass kernel for MultiHeadAttention + residual + BatchNorm.

Model (reference):
  q = query @ Wq.T ; k = key @ Wk.T ; v = key @ Wv.T    (per-head split)
  score = q k^T / sqrt(D), causal mask, softmax over keys
  res   = (attn @ v) + query
  out   = batchnorm(res over all (N*L) rows, per feature) * gamma + beta

Sharding over 8 cores: FEATURE sharding. Core c owns heads {2c, 2c+1}
(features [128c, 128c+128)) for ALL batches. BatchNorm statistics are
then core-local, so no collective is needed at all.

All-fp8 attention datapath: projections use fp8 DoubleRow; projected
q/k are re-quantized to fp8 and DMA-shuffled into a [32-part, 2-pair]
layout (the host permutes Wq/Wk columns so each shuffle is 2 full-width
DMAs) so the score matmuls also run as fp8 DoubleRow (half the PE
columns).  Attention weights are produced in fp8 by ACT (exp writes
fp8 directly) and DVE (e4m3 *bits* via an int8 Schraudolph
scalar_tensor_tensor - valid because |score*scale| is small so the
biased-exponent byte is always in [38, 90]).  Full key blocks are
exp'd pairwise into one [128, 2048] fp8 tile so the attention@V
matmuls run as fp8 DoubleRow over 256 keys at a time.  V carries an
appended constant column whose products accumulate into per-(ic,n,h,S)
columns of a once-cleared PSUM bank, giving softmax denominators
without extra PE passes.

Engine budget: scores/AV/projections on PE; exp split ACT/DVE by a
greedy cost balancer; masks + residual-adds on GPSIMD (which cannot
touch PSUM); DVE carries the PSUM-coupled glue (re-quantize copies,
normalize, reciprocal, batch-norm).  PSUM is exactly packed: 6 score
half-slots (shared with projection accumulators) + attention-value
bank + denominator/stats bank.
"""

import math
import sys

sys.path.insert(0, "/opt/trn_rl_repo")

import numpy as np
import ml_dtypes

import concourse.bass as bass
import concourse.mybir as mybir
from concourse import bacc
import concourse.tile as tile

F32 = mybir.dt.float32
BF16 = mybir.dt.bfloat16
FP8 = mybir.dt.float8e4
I8 = mybir.dt.int8
BF16_NP = ml_dtypes.bfloat16
FP8_NP = mybir.dt.np(FP8)
# q/k weights are scaled by 16 on the host so fp8e4 stays out of the
# subnormal range.
W8_SCALE = 16.0

N = 4
L = 2048
D = 1024
H = 16
P = 64
NCORES = 8
FC = D // NCORES       # features per core = 128
H2 = 2                 # heads per core
EPS = 1e-5
SCALE = 1.0 / math.sqrt(D)
NL = N * L             # 8192 rows in the global batch norm

# fp8 staging scales: q8 = (16 q) * SQK = 8 q ; k8 likewise ; v8 = 8 v.
SQK = 0.5
SV = 8.0
VONES = 8.0            # denominator column value (folded out by 1/(8*sum))
# score PSUM = sum q8 k8 = 64 * raw score -> exp argument scale:
EXP_SCALE = SCALE / 64.0
# e4m3-bit Schraudolph: i8 = round(st2 * A8V + B8V) gives bits of
# ~exp(st2 * EXP_SCALE)
A8V = (8.0 / math.log(2.0)) * EXP_SCALE
B8V = 56.0 - 0.3494 + 0.5

# static per-unit engine-busy costs (ns) for the greedy load balancer.
# NOTE: the Pool/GPSIMD engine cannot access PSUM, so only SBUF-input
# work (masks, the drain residual-add) is Pool-eligible.
COST_FULL_HALF = {"A": 612, "D": 631}
COST_DIAG_HALF = [{"A": 612, "D": 631}, {"A": 505, "D": 504},
                  {"A": 398, "D": 376}, {"A": 292, "D": 247}]
COST_MASK = {"D": 1127, "P": 2126}      # merged, one per (ic, n)
COST_DRAIN = {"D": 523}                 # recip + normalize TT (PSUM: DVE)
COST_DRAIN_ADD = {"D": 127, "P": 603}   # residual add (SBUF)
COST_DSQ = {"D": 193, "P": 1110}        # res^2 for the variance (SBUF)
COST_VCOPY = {"A": 292, "D": 247}
COST_QKCOPY = {"D": 631, "A": 668}

# software-pipeline stage lags
EXP_LAG = 1
MASK_LAG = 4
AV_LAG = 8
DEBUG_NOLAG = False
# force a spec's two exp halves onto different engines (overlap the score
# slot-pair release) only while the engines are within this busy gap (ns);
# beyond it, let the balancer pile both halves on the lighter engine.
EXCL_GAP = 1000

_cached = {}


def build_program(l=L):
    """Build the SPMD Bass program (identical on all 8 cores)."""
    nc = bacc.Bacc("TRN2", target_bir_lowering=False, debug=False,
                   num_devices=NCORES)

    ic_n = l // 512        # 512-row query chunks
    nlc = N * l            # rows per core (all batches)

    xq8_nd = nc.dram_tensor("xq8_nd", [D, nlc], FP8,
                            kind="ExternalInput").ap()
    xk8_nd = nc.dram_tensor("xk8_nd", [D, nlc], FP8,
                            kind="ExternalInput").ap()
    wq8 = nc.dram_tensor("wq8", [D, FC], FP8, kind="ExternalInput").ap()
    wk8 = nc.dram_tensor("wk8", [D, FC], FP8, kind="ExternalInput").ap()
    wvt = nc.dram_tensor("wvt", [D, FC], FP8, kind="ExternalInput").ap()
    # residual input, pre-blocked on the host to the SBUF res layout:
    # [row-in-block, (n, ic*4+S, f)]
    q_res = nc.dram_tensor("q_res", [128, (nlc // 128) * FC], BF16,
                           kind="ExternalInput").ap()
    gamma = nc.dram_tensor("gamma", [1, FC], F32, kind="ExternalInput").ap()
    beta = nc.dram_tensor("beta", [1, FC], F32, kind="ExternalInput").ap()
    # output keeps the SBUF res layout: [row-in-block, (n, ic, S, f)] so the
    # store is one fat contiguous DMA per batch; the host untangles it.
    out_s = nc.dram_tensor("out_s", [128, (nlc // 128) * FC], BF16,
                           kind="ExternalOutput").ap()

    jblocks = l // 128     # 16 key blocks per batch
    nls = nlc // 128       # 64 ls blocks of res

    from contextlib import ExitStack
    with ExitStack() as stack:
        tc = stack.enter_context(tile.TileContext(nc))
        pool = {}
        for nm, bufs, space in (
                ("consts", 1, None), ("persist", 1, None), ("wt", 1, None),
                ("qtp", 2, None), ("xq", 6, None), ("xk8", 6, None),
                ("qf8", 3, None), ("kf8", 3, None),
                ("at8", 16, None), ("at8d", 4, None),
                ("sq", 4, None),
                ("outp", 2, None), ("bnp", 1, None), ("small", 10, None),
                ("st2", 6, "PSUM"), ("av", 1, "PSUM"),
                ("stat", 1, "PSUM")):
            kw = {"name": nm, "bufs": bufs}
            if space:
                kw["space"] = space
            pool[nm] = stack.enter_context(tc.tile_pool(**kw))
        consts, persist, wtp = pool["consts"], pool["persist"], pool["wt"]
        qtp, xqp, xk8p = pool["qtp"], pool["xq"], pool["xk8"]
        qf8p, kf8p = pool["qf8"], pool["kf8"]
        at8p, at8dp = pool["at8"], pool["at8d"]
        sqp = pool["sq"]
        outp, bnp, smallp = pool["outp"], pool["bnp"], pool["small"]
        st2p, avp, statp = pool["st2"], pool["av"], pool["stat"]

        # greedy engine balancer: assign each elementwise unit to the
        # engine with the least projected busy time.  Hard-assigned work
        # is debited at emission time so the balance tracks temporally.
        eng_busy = {"A": 2600.0, "D": 0.0, "P": 1500.0}
        engs = {}

        def pick(costs, exclude=None):
            elig = {k: v for k, v in costs.items() if k != exclude}
            e = min(elig, key=lambda k: eng_busy[k] + elig[k])
            eng_busy[e] += costs[e]
            return e

        def eng_of(e):
            if not engs:
                engs.update({"A": nc.scalar, "D": nc.vector,
                             "P": nc.gpsimd})
            return engs[e]

        # ---------------- first activation chunk DMAs (critical path) ----
        def x8_src(dram, n, ic):
            # d = s*256 + t*128 + p (fp8 DoubleRow pair layout)
            return bass.AP(
                tensor=dram.tensor,
                offset=dram.offset + n * l + ic * 512,
                ap=[[nlc, 128], [256 * nlc, 4], [128 * nlc, 2], [1, 512]])

        def load_x8(pool_, dram, n, ic):
            t = pool_.tile([128, 4 * 2 * 512], FP8, tag="x8", name="x8t")
            nc.sync.dma_start(
                t.rearrange("p (s t x) -> p s t x", s=4, t=2),
                x8_src(dram, n, ic))
            return t

        wts = {}

        def load_w8(wname, wdram):
            t = wtp.tile([128, 4 * 2 * FC], FP8, tag=wname, name=wname)
            nc.sync.dma_start(
                t.rearrange("p (s t f) -> p s t f", s=4, t=2),
                bass.AP(tensor=wdram.tensor, offset=wdram.offset,
                        ap=[[FC, 128], [256 * FC, 4], [128 * FC, 2],
                            [1, FC]]))
            wts[wname] = t

        load_w8("wq", wq8)
        xq_t = load_x8(xqp, xq8_nd, 0, 0)
        load_w8("wk", wk8)
        xk8_t = load_x8(xk8p, xk8_nd, 0, 0)
        load_w8("wv", wvt)

        # ---------------- constants -------------------------------------
        ones_col = consts.tile([128, 1], BF16)
        nc.vector.memset(ones_col, 1.0)
        eps_sb = consts.tile([128, 1], F32)
        nc.vector.memset(eps_sb, EPS)
        gamma_sb = consts.tile([1, FC], F32)
        nc.sync.dma_start(gamma_sb, gamma)
        beta_sb = consts.tile([1, FC], F32)
        nc.sync.dma_start(beta_sb, beta)
        # lower-triangular (j <= i) mask in [j-part, i-free] layout, fp8
        tm_f = consts.tile([128, 128], F32)
        nc.vector.memset(tm_f, 1.0)
        nc.gpsimd.affine_select(
            out=tm_f, in_=tm_f,
            compare_op=mybir.AluOpType.is_ge, fill=0.0, base=0,
            pattern=[[1, 128]], channel_multiplier=-1)
        trimask = consts.tile([128, 128], FP8)
        nc.vector.tensor_copy(trimask, tm_f)
        # zero operands for the PSUM-bank-clearing matmuls
        zlhs = consts.tile([128, 128], BF16)
        nc.vector.memset(zlhs, 0.0)
        zrhs = consts.tile([128, 512], BF16)
        nc.vector.memset(zrhs, 0.0)
        b8_sb = consts.tile([128, 1], F32)
        nc.vector.memset(b8_sb, B8V)

        def b8bc(*freedims):
            ap = [[b8_sb.ap[0][0], 128]] + [[0, d] for d in freedims]
            return bass.AP(tensor=b8_sb.tensor, offset=b8_sb.offset, ap=ap)

        # ---------------- persistent SBUF -------------------------------
        # kt8_sb: [32p' (x2 heads), (t, n, j)] fp8 DoubleRow pair layout:
        # feature d of head h = h*64 + t*32 + p' lives at partition
        # h*32 + p', column block t.
        kt8_sb = persist.tile([64, 2 * N * l], FP8, tag="kt8")
        # v_sb: [j-in-block, (n, jc, h, 65)] fp8, x8 scaled; col 64 of each
        # 65-group is the baked denominator column (value 8.0)
        v_sb = persist.tile([128, N * jblocks * H2 * 65], FP8, tag="v")
        v3 = v_sb.rearrange("p (g x) -> p g x", x=65)
        nc.gpsimd.memset(v3[:, :, 64:65], VONES)
        # res_sb: [l-in-block, (ls, f)] bf16, ls = n*16 + ic*4 + S
        res_sb = persist.tile([128, nls * FC], BF16, tag="res")
        # qres_sb: the residual input, loaded once: [128, (n, ics, f)]
        qres_sb = persist.tile([128, nls * FC], BF16, tag="qres")

        # statdn bank (2KB, cleared once up front, then start=False only):
        #   cols [0,256): softmax denominator accumulators, one column per
        #                 (ic, n, h, S) - no reuse, so no re-clearing
        #   cols [256,384): per-feature sums (partition 0)
        #   cols [384,512): per-feature sums of squares (partition 0)
        statdn = statp.tile([128, 512], F32, tag="statdn")
        nc.tensor.matmul(statdn, zlhs, zrhs, start=True, stop=True,
                         skip_group_check=True)

        def dn_col(ic, n, h, S):
            return (n * 4 + ic) * 8 + h * 4 + S

        def load_qres(n):
            nc.sync.dma_start(
                qres_sb[:, n * 16 * FC:(n + 1) * 16 * FC],
                bass.AP(tensor=q_res.tensor,
                        offset=q_res.offset + n * 16 * FC,
                        ap=[[nls * FC, 128], [1, 16 * FC]]))

        # ------------------------------------------------------------------
        # projection task machinery (filler micro-ops paced into B loops)
        # ------------------------------------------------------------------
        # projection chains rotate through the same PSUM pool as the score
        # tiles (6 x 2KB bank-slots)
        def alloc_pj():
            return st2p.tile([128, 512], F32, tag="st2", name="pj")

        chain_alloc = {"fn": alloc_pj}

        # fp8 staging tiles for the partition shuffle, per (ic, half)
        qf8_cur = {}
        kf8_cur = {}

        def qk_chain(side, n, ic, get_xt):
            """q/k projection via fp8 DoubleRow, then re-quantize to fp8
            into the shuffle staging tile."""
            w_use = wts["wq"] if side == "q" else wts["wk"]
            w4 = w_use.rearrange("p (s t f) -> p s t f", s=4, t=2)
            pj = {}
            alloc_fn = chain_alloc["fn"]

            def alloc():
                pj["t"] = alloc_fn()

            def mm(s0):
                x4 = get_xt().rearrange("p (s t x) -> p s t x", s=4, t=2)
                for s in (s0, s0 + 1):
                    nc.tensor.matmul(
                        pj["t"], w4[:, s], x4[:, s],
                        start=(s == 0), stop=(s == 3),
                        perf_mode=mybir.MatmulPerfMode.DoubleRow)

            def copy():
                stage = qf8_cur if side == "q" else kf8_cur
                key = (ic, n) if ic == 0 else (ic, n // 2)
                if key not in stage:
                    p_ = qf8p if side == "q" else kf8p
                    nb = 1 if ic == 0 else 2
                    stage[key] = (p_.tile([128, nb * 512], FP8, tag="qf8",
                                          name=f"{side}f8"), nb)
                t, nb = stage[key]
                off = 0 if ic == 0 else (n % 2) * 512
                e = pick(COST_QKCOPY)
                if e == "A":
                    nc.scalar.activation(t[:, off:off + 512], pj["t"],
                                         mybir.ActivationFunctionType.Copy,
                                         scale=SQK)
                else:
                    nc.vector.tensor_scalar_mul(t[:, off:off + 512],
                                                pj["t"], SQK)

            ops = [alloc]
            for s0 in range(0, 4, 2):
                ops.append(lambda s=s0: mm(s))
            ops.append(copy)
            return ops

        def v_chain(n, jsub, ic, get_xt):
            pj = {}
            alloc_fn = chain_alloc["fn"]
            wv4 = wts["wv"].rearrange("p (s t f) -> p s t f", s=4, t=2)

            def alloc():
                pj["t"] = alloc_fn()

            def mm(s0):
                x4 = get_xt().rearrange("p (s t x) -> p s t x", s=4, t=2)
                for s in (s0, s0 + 1):
                    nc.tensor.matmul(
                        pj["t"][:, 0:128],
                        x4[:, s, :, jsub * 128:jsub * 128 + 128],
                        wv4[:, s],
                        start=(s == 0), stop=(s == 3),
                        perf_mode=mybir.MatmulPerfMode.DoubleRow)

            def copy():
                jc = ic * 4 + jsub
                base = (n * jblocks + jc) * H2 * 65
                dst = v_sb[:, base:base + 130].rearrange(
                    "p (h x) -> p h x", h=2)[:, :, 0:64]
                src = pj["t"][:, 0:128].rearrange("p (h x) -> p h x", h=2)
                e = pick(COST_VCOPY)
                if e == "A":
                    nc.scalar.activation(dst, src,
                                         mybir.ActivationFunctionType.Copy,
                                         scale=SV)
                else:
                    nc.vector.tensor_scalar_mul(dst, src, SV)

            ops = [alloc]
            for s0 in range(0, 4, 2):
                ops.append(lambda s=s0: mm(s))
            ops.append(copy)
            return ops

        # qt8 per chunk: [32p' (x2 heads), (t, n, i)] fp8 pair layout
        qt_tiles = {}

        def emit_shuffles(ic, key, batches):
            """DMA the staged q/k fp8 tiles into DoubleRow pair layout.
            The host permutes Wq/Wk columns so projection PSUM partition
            m = t*64 + h*32 + p' holds feature h*64 + t*32 + p'; each t
            half then shuffles with one full-width DMA per side."""
            qt8 = qt_tiles[ic]
            q_stage = qf8_cur.pop(key)[0]
            k_stage = kf8_cur.pop(key)[0]
            nb = len(batches)
            n0 = batches[0]
            for t in range(2):
                src = q_stage[t * 64:(t + 1) * 64]
                dst = qt8[0:64,
                          t * (N * 512) + n0 * 512:
                          t * (N * 512) + n0 * 512 + nb * 512]
                nc.sync.dma_start(dst, src)
            for t in range(2):
                src = k_stage[t * 64:(t + 1) * 64]
                if nb > 1:
                    src = src.rearrange("p (u i) -> p u i", u=nb)
                    dst = bass.AP(
                        tensor=kt8_sb.tensor,
                        offset=kt8_sb.offset + t * (N * l) + n0 * l
                        + ic * 512,
                        ap=[[kt8_sb.ap[0][0], 64], [l, nb], [1, 512]])
                else:
                    dst = kt8_sb[0:64,
                                 t * (N * l) + n0 * l + ic * 512:
                                 t * (N * l) + n0 * l + ic * 512 + 512]
                nc.sync.dma_start(dst, src)

        def build_chunk_groups(ic, first_x):
            """Return per-batch lists of micro-op closures for A(ic)."""
            groups = []
            xq_cur = {0: first_x[0]}
            xk8_cur = {0: first_x[1]}
            for n in range(N):
                chain_alloc["fn"] = alloc_pj
                ops = []
                get_xq = lambda nn=n: xq_cur[nn]
                get_xk8 = lambda nn=n: xk8_cur[nn]
                if n + 1 < N:
                    def pre(nn=n + 1):
                        xq_cur[nn] = load_x8(xqp, xq8_nd, nn, ic)
                        xk8_cur[nn] = load_x8(xk8p, xk8_nd, nn, ic)
                    ops.append(pre)
                if ic == 0:
                    ops.append(lambda nn=n: load_qres(nn))
                ops += qk_chain("q", n, ic, get_xq)
                ops += qk_chain("k", n, ic, get_xk8)
                # shuffle once the staging tile for this group is complete
                if ic == 0:
                    ops.append(lambda nn=n: emit_shuffles(0, (0, nn), [nn]))
                elif n % 2 == 1:
                    ops.append(lambda nn=n: emit_shuffles(
                        ic, (ic, nn // 2), [nn - 1, nn]))
                for jsub in range(4):
                    ops += v_chain(n, jsub, ic, get_xk8)
                groups.append(ops)
            return groups

        # ------------------------------------------------------------------
        # A(0): batch 0's projections + its shuffle run up front; the rest
        # is deadline-paced into B(0).
        # ------------------------------------------------------------------
        qt_tiles[0] = qtp.tile([64, 2 * N * 512], FP8, tag="qt", name="qt")
        groups0 = build_chunk_groups(0, (xq_t, xk8_t))
        for op in groups0[0]:
            op()

        # ------------------------------------------------------------------
        # main loop: one software-pipelined stream over (ic, n, jc).
        # ------------------------------------------------------------------
        specs = []
        for ic in range(ic_n):
            for n in range(N):
                for jc in range(4 * ic + 4):
                    specs.append((ic, n, jc))
        nspec = len(specs)
        st2_of, pair_of, diag_of, avs_of = {}, {}, {}, {}
        sched = []

        def schedule_group(ops, w_start, w_end):
            no = len(ops)
            span = max(1, w_end - w_start)
            for k, op in enumerate(ops):
                sched.append((w_start + (k * span) // no, op))

        def emit_due(idx):
            while sched and sched[0][0] <= idx:
                sched.pop(0)[1]()

        def stage_scores(idx):
            ic, n, jc = specs[idx]
            qt8 = qt_tiles[ic]
            ts = []
            for h in range(H2):
                st2 = st2p.tile([128, 512], F32, tag="st2", name="st2")
                ts.append(st2)
                lhsT = bass.AP(
                    tensor=kt8_sb.tensor,
                    offset=kt8_sb.offset + (h * 32) * kt8_sb.ap[0][0]
                    + n * l + jc * 128,
                    ap=[[kt8_sb.ap[0][0], 32], [N * l, 2], [1, 128]])
                rhs = bass.AP(
                    tensor=qt8.tensor,
                    offset=qt8.offset + (h * 32) * qt8.ap[0][0] + n * 512,
                    ap=[[qt8.ap[0][0], 32], [N * 512, 2], [1, 512]])
                nc.tensor.matmul(
                    st2, lhsT, rhs, start=True, stop=True,
                    perf_mode=mybir.MatmulPerfMode.DoubleRow)
            st2_of[idx] = ts

        def stage_exp(idx):
            ic, n, jc = specs[idx]
            rr = jc - 4 * ic
            sts = st2_of.pop(idx)
            if rr < 0:
                m = jc // 2
                key = (ic, n, m)
                if key not in pair_of:
                    pair_of[key] = at8p.tile([128, 2048], FP8, tag="at8",
                                             name="at8")
                base = (jc % 2) * 1024
                prev = None
                for h in range(H2):
                    dst = pair_of[key][:, base + h * 512:base + h * 512 + 512]
                    gap = abs(eng_busy["A"] - eng_busy["D"])
                    eng = pick(COST_FULL_HALF,
                               exclude=(prev if gap < EXCL_GAP else None))
                    prev = eng
                    if eng == "A":
                        nc.scalar.activation(
                            dst, sts[h], mybir.ActivationFunctionType.Exp,
                            scale=EXP_SCALE)
                    else:
                        eng_of(eng).scalar_tensor_tensor(
                            out=dst.bitcast(I8), in0=sts[h], scalar=A8V,
                            in1=b8bc(512),
                            op0=mybir.AluOpType.mult, op1=mybir.AluOpType.add)
            else:
                if rr == 0:
                    diag_of[(ic, n)] = at8dp.tile([128, 4096], FP8,
                                                  tag="at8d", name="at8d")
                t = diag_of[(ic, n)]
                w = (4 - rr) * 128
                prev = None
                for h in range(H2):
                    dst = t[:, rr * 1024 + h * 512 + rr * 128:
                            rr * 1024 + (h + 1) * 512]
                    src = sts[h][:, rr * 128:512]
                    gap = abs(eng_busy["A"] - eng_busy["D"])
                    eng = pick(COST_DIAG_HALF[rr],
                               exclude=(prev if gap < EXCL_GAP else None))
                    prev = eng
                    if eng == "A":
                        nc.scalar.activation(
                            dst, src, mybir.ActivationFunctionType.Exp,
                            scale=EXP_SCALE)
                    else:
                        eng_of(eng).scalar_tensor_tensor(
                            out=dst.bitcast(I8), in0=src, scalar=A8V,
                            in1=b8bc(w),
                            op0=mybir.AluOpType.mult, op1=mybir.AluOpType.add)

        def stage_mask(idx):
            # one merged mask multiply per (ic, n): the 4 diagonal blocks'
            # masked squares live at col rr*1152 + h*512 in the shared tile
            ic, n, jc = specs[idx]
            rr = jc - 4 * ic
            if rr != 3:
                return
            t = diag_of[(ic, n)]
            sl = bass.AP(tensor=t.tensor, offset=t.offset,
                         ap=[[t.ap[0][0], 128], [1152, 4], [512, 2],
                             [1, 128]])
            tm = bass.AP(tensor=trimask.tensor, offset=trimask.offset,
                         ap=[[trimask.ap[0][0], 128], [0, 4], [0, 2],
                             [1, 128]])
            eng_of(pick(COST_MASK)).tensor_mul(sl, sl, tm)

        def stage_av(idx):
            ic, n, jc = specs[idx]
            rr = jc - 4 * ic
            if jc == 0:
                avv = avp.tile([128, 512], F32, tag="avv", name="avv")
                avs_of[(ic, n)] = avv
                nc.tensor.matmul(avv, zlhs, zrhs, start=True, stop=True,
                                 skip_group_check=True)
            avv = avs_of[(ic, n)]
            if rr < 0:
                if jc % 2 == 0:
                    return
                pair = pair_of.pop((ic, n, jc // 2))
                vbase = (n * jblocks + (jc - 1)) * H2 * 65
                for h in range(H2):
                    vv = bass.AP(
                        tensor=v_sb.tensor,
                        offset=v_sb.offset + vbase + h * 65,
                        ap=[[v_sb.ap[0][0], 128], [H2 * 65, 2], [1, 64]])
                    vo = bass.AP(
                        tensor=v_sb.tensor,
                        offset=v_sb.offset + vbase + h * 65 + 64,
                        ap=[[v_sb.ap[0][0], 128], [H2 * 65, 2], [1, 1]])
                    for S in range(4):
                        lhsT = bass.AP(
                            tensor=pair.tensor,
                            offset=pair.offset + h * 512 + S * 128,
                            ap=[[pair.ap[0][0], 128], [1024, 2], [1, 128]])
                        nc.tensor.matmul(
                            avv[:, h * 256 + S * 64:h * 256 + S * 64 + 64],
                            lhsT, vv, start=False, stop=False,
                            perf_mode=mybir.MatmulPerfMode.DoubleRow,
                            skip_group_check=True)
                        c = dn_col(ic, n, h, S)
                        nc.tensor.matmul(
                            statdn[:, c:c + 1], lhsT, vo,
                            start=False, stop=False,
                            perf_mode=mybir.MatmulPerfMode.DoubleRow,
                            skip_group_check=True)
            else:
                t = diag_of[(ic, n)] if rr < 3 else diag_of.pop((ic, n))
                vbase = (n * jblocks + jc) * H2 * 65
                for h in range(H2):
                    for S in range(4):
                        if rr > S:
                            continue
                        lhsT = t[:, rr * 1024 + h * 512 + S * 128:
                                 rr * 1024 + h * 512 + S * 128 + 128]
                        nc.tensor.matmul(
                            avv[:, h * 256 + S * 64:h * 256 + S * 64 + 64],
                            lhsT,
                            v_sb[:, vbase + h * 65:vbase + h * 65 + 64],
                            start=False, stop=(rr == S),
                            skip_group_check=True)
                        c = dn_col(ic, n, h, S)
                        nc.tensor.matmul(
                            statdn[:, c:c + 1], lhsT,
                            v_sb[:, vbase + h * 65 + 64:vbase + h * 65 + 65],
                            start=False, stop=(rr == S),
                            skip_group_check=True)
            if jc == 4 * ic + 3:
                enqueue_drain(ic, n)

        # drains and stats run as small deferred pieces, one per iteration
        drain_pending = []

        def enqueue_drain(ic, n):
            avv = avs_of.pop((ic, n))
            base512 = (n * 16 + ic * 4) * FC

            def drain_head(h):
                c0 = dn_col(ic, n, h, 0)
                rec = smallp.tile([128, 4], F32, tag="rec", name="rec")
                nc.vector.reciprocal(rec, statdn[:, c0:c0 + 4])
                eng_busy["D"] += COST_DRAIN["D"]

                def s4(t, off, inner=64):
                    return bass.AP(tensor=t.tensor, offset=t.offset + off,
                                   ap=[[t.ap[0][0], 128], [FC, 4],
                                       [1, inner]])

                # normalized = avv * (1/denom), broadcast per S group
                nt = smallp.tile([128, 256], BF16, tag="nt", name="nt")
                nc.vector.tensor_mul(
                    nt.rearrange("p (s x) -> p s x", s=4),
                    bass.AP(tensor=avv.tensor,
                            offset=avv.offset + h * 256,
                            ap=[[avv.ap[0][0], 128], [64, 4], [1, 64]]),
                    bass.AP(tensor=rec.tensor, offset=rec.offset,
                            ap=[[rec.ap[0][0], 128], [1, 4], [0, 64]]))
                # + residual, strided into res_sb feature slots
                eng_of(pick(COST_DRAIN_ADD)).tensor_add(
                    s4(res_sb, base512 + h * 64),
                    nt.rearrange("p (s x) -> p s x", s=4),
                    s4(qres_sb, base512 + h * 64))

            def drain_sq():
                res_block = res_sb[:, base512:base512 + 512]
                sqt = sqp.tile([128, 512], BF16, tag="sq", name="sqt")
                eng_of(pick(COST_DSQ)).tensor_mul(sqt, res_block, res_block)
                stats_bufs[(ic, n)] = (res_block, sqt)

            def drain_stats():
                res_block, sqt = stats_bufs.pop((ic, n))
                for g in range(4):
                    last = (n == N - 1 and ic == ic_n - 1 and g == 3)
                    nc.tensor.matmul(statdn[0:1, 256:256 + FC], ones_col,
                                     res_block[:, g * FC:(g + 1) * FC],
                                     start=False, stop=last,
                                     skip_group_check=True)
                    nc.tensor.matmul(statdn[0:1, 384:384 + FC], ones_col,
                                     sqt[:, g * FC:(g + 1) * FC],
                                     start=False, stop=last,
                                     skip_group_check=True)

            if DEBUG_NOLAG:
                drain_head(0)
                drain_head(1)
                drain_sq()
                drain_stats()
            else:
                drain_pending.extend(
                    [lambda: drain_head(0), None,
                     lambda: (drain_head(1), drain_sq()), None, drain_stats])

        stats_bufs = {}

        def step_drain(flush=False):
            while drain_pending:
                op = drain_pending.pop(0)
                if op is None:
                    if flush:
                        continue
                    return
                op()

        # spec index of (ic, n, jc=0), for filler deadlines
        start_idx = {}
        for i, (sic, sn, sjc) in enumerate(specs):
            if sjc == 0:
                start_idx[(sic, sn)] = i

        # chunk 0's remaining groups are due just before B(0) reaches
        # batch 2 (their shuffle covers batches 1-3)
        for n in range(1, N):
            schedule_group(groups0[n], start_idx[(0, n - 1)],
                           max(1, start_idx[(0, n)] - 1))

        for idx in range(nspec + max(EXP_LAG, MASK_LAG, AV_LAG)):
            if idx < nspec:
                ic, n, jc = specs[idx]
                if jc == 0 and n == 0 and ic + 1 < ic_n:
                    qt_tiles[ic + 1] = qtp.tile([64, 2 * N * 512], FP8,
                                                tag="qt", name="qt")
                    nxq = load_x8(xqp, xq8_nd, 0, ic + 1)
                    nxk8 = load_x8(xk8p, xk8_nd, 0, ic + 1)
                    groups = build_chunk_groups(ic + 1, (nxq, nxk8))
                    # group g due just before B(ic+1) needs its shuffle:
                    # shuffles fire at the end of groups 1 and 3, needed at
                    # B(ic+1) batches 0 and 2 respectively.
                    w_prev = idx
                    deadlines = [
                        (idx + start_idx[(ic + 1, 0)]) // 2,
                        max(1, start_idx[(ic + 1, 0)] - 1),
                        start_idx[(ic + 1, 1)],
                        max(1, start_idx[(ic + 1, 2)] - 1)]
                    for g in range(N):
                        w_end = deadlines[g]
                        schedule_group(groups[g], w_prev, w_end)
                        w_prev = w_end
                stage_scores(idx)
            if idx - EXP_LAG >= 0 and idx - EXP_LAG < nspec:
                stage_exp(idx - EXP_LAG)
            if idx - MASK_LAG >= 0 and idx - MASK_LAG < nspec:
                stage_mask(idx - MASK_LAG)
            step_drain()
            if idx - AV_LAG >= 0 and idx - AV_LAG < nspec:
                stage_av(idx - AV_LAG)
            emit_due(idx)
        while sched:
            sched.pop(0)[1]()
        step_drain(flush=True)

        # ------------------------------------------------------------------
        # batch-norm: compute gamma', beta', apply
        # ------------------------------------------------------------------
        sumf = statdn[0:1, 256:256 + FC]
        sqf = statdn[0:1, 384:384 + FC]

        inv = 1.0 / NL
        mean = bnp.tile([1, FC], F32, tag="mean", name="mean")
        nc.vector.tensor_scalar_mul(mean, sumf, inv)
        musq = bnp.tile([1, FC], F32, tag="musq", name="musq")
        nc.vector.tensor_mul(musq, mean, mean)
        var = bnp.tile([1, FC], F32, tag="var", name="var")
        nc.vector.scalar_tensor_tensor(
            out=var, in0=sqf, scalar=inv, in1=musq,
            op0=mybir.AluOpType.mult, op1=mybir.AluOpType.subtract)
        std = bnp.tile([1, FC], F32, tag="std", name="std")
        nc.scalar.activation(std, var, mybir.ActivationFunctionType.Sqrt,
                             bias=eps_sb[0:1, :])
        rstd = bnp.tile([1, FC], F32, tag="rstd", name="rstd")
        nc.vector.reciprocal(rstd, std)
        gp = bnp.tile([1, FC], F32, tag="gp", name="gp")
        nc.vector.tensor_mul(gp, gamma_sb, rstd)
        mgp = bnp.tile([1, FC], F32, tag="mgp", name="mgp")
        nc.vector.tensor_mul(mgp, mean, gp)
        bp = bnp.tile([1, FC], F32, tag="bp", name="bp")
        nc.vector.tensor_sub(bp, beta_sb, mgp)
        gp16 = bnp.tile([1, FC], BF16, tag="gp16", name="gp16")
        nc.vector.tensor_copy(gp16, gp)
        bp16 = bnp.tile([1, FC], BF16, tag="bp16", name="bp16")
        nc.vector.tensor_copy(bp16, bp)

        gbc = bnp.tile([128, FC], BF16, tag="gbc", name="gbc")
        nc.gpsimd.partition_broadcast(gbc, gp16)
        bbc = bnp.tile([128, FC], BF16, tag="bbc", name="bbc")
        nc.gpsimd.partition_broadcast(bbc, bp16)

        def rep16(t):
            return bass.AP(tensor=t.tensor, offset=t.offset,
                           ap=[[t.ap[0][0], 128], [0, 16], [1, FC]])

        for n in range(N):
            base = n * 16 * FC
            t1 = outp.tile([128, 16 * FC], BF16, tag="t1", name="t1")
            nc.vector.tensor_mul(t1.rearrange("p (g f) -> p g f", g=16),
                                 res_sb[:, base:base + 16 * FC].rearrange(
                                     "p (g f) -> p g f", g=16),
                                 rep16(gbc))
            ot = outp.tile([128, 16 * FC], BF16, tag="ot", name="ot")
            nc.vector.tensor_add(ot.rearrange("p (g f) -> p g f", g=16), t1.rearrange(
                "p (g f) -> p g f", g=16), rep16(bbc))
            nc.sync.dma_start(
                bass.AP(tensor=out_s.tensor,
                        offset=out_s.offset + n * (16 * FC),
                        ap=[[nls * FC, 128], [1, 16 * FC]]),
                ot)

    nc.compile()
    return nc


def get_runner(nc):
    """Build (once) a cached jitted SPMD executor for the Bass program."""
    if "runner" in _cached:
        return _cached["runner"]

    import jax
    from jax.experimental.shard_map import shard_map
    from jax.sharding import Mesh, PartitionSpec
    from concourse import bass2jax

    bass2jax.install_neuronx_cc_hook()

    partition_name = (nc.partition_id_tensor.name
                      if nc.partition_id_tensor else None)
    in_names, out_names, out_avals, zero_outs = [], [], [], []
    for alloc in nc.m.functions[0].allocations:
        if not isinstance(alloc, mybir.MemoryLocationSet):
            continue
        name = alloc.memorylocations[0].name
        if alloc.kind == "ExternalInput":
            if name != partition_name:
                in_names.append(name)
        elif alloc.kind == "ExternalOutput":
            shape = tuple(alloc.tensor_shape)
            dtype = mybir.dt.np(alloc.dtype)
            out_names.append(name)
            out_avals.append(jax.core.ShapedArray(shape, dtype))
            zero_outs.append(np.zeros(shape, dtype))
    n_params = len(in_names)
    n_outs = len(out_avals)
    all_names = in_names + out_names
    if partition_name is not None:
        all_names = all_names + [partition_name]

    def _body(*args):
        operands = list(args)
        if partition_name is not None:
            operands.append(bass2jax.partition_id_tensor())
        outs = bass2jax._bass_exec_p.bind(
            *operands,
            out_avals=tuple(out_avals),
            in_names=tuple(all_names),
            out_names=tuple(out_names),
            lowering_input_output_aliases=(),
            sim_require_finite=True,
            sim_require_nnan=True,
            nc=nc,
        )
        return tuple(outs)

    devices = jax.devices()[:NCORES]
    mesh = Mesh(np.asarray(devices), ("core",))
    in_specs = (PartitionSpec("core"),) * (n_params + n_outs)
    out_specs = (PartitionSpec("core"),) * n_outs
    donate = tuple(range(n_params, n_params + n_outs))
    sharded = jax.jit(
        shard_map(_body, mesh=mesh, in_specs=in_specs, out_specs=out_specs,
                  check_rep=False),
        donate_argnums=donate, keep_unused=True)

    def run_np(in_maps):
        concat_in = [
            np.concatenate([np.asarray(in_maps[c][nm]) for c in range(NCORES)],
                           axis=0)
            for nm in in_names]
        concat_zeros = [np.zeros((NCORES * z.shape[0], *z.shape[1:]), z.dtype)
                        for z in zero_outs]
        out_arrs = sharded(*concat_in, *concat_zeros)
        return [
            {nm: np.asarray(out_arrs[i]).reshape(
                NCORES, *out_avals[i].shape)[c]
             for i, nm in enumerate(out_names)}
            for c in range(NCORES)]

    _cached["runner"] = (run_np, sharded, in_names, out_names, out_avals,
                         zero_outs, mesh)
    return _cached["runner"]


def make_in_maps(inputs, l):
    query = np.asarray(inputs["query"], dtype=np.float32)
    key = np.asarray(inputs["key"], dtype=np.float32)
    Wq = np.asarray(inputs["Wq"], dtype=np.float32)
    Wk = np.asarray(inputs["Wk"], dtype=np.float32)
    Wv = np.asarray(inputs["Wv"], dtype=np.float32)
    gamma = np.asarray(inputs["gamma"], dtype=np.float32)
    beta = np.asarray(inputs["beta"], dtype=np.float32)

    n = query.shape[0]
    qf = query.reshape(n * l, D)
    kf = key.reshape(n * l, D)
    xq8 = np.ascontiguousarray(qf.T.astype(FP8_NP))
    xk8 = np.ascontiguousarray(kf.T.astype(FP8_NP))

    # Wq/Wk output-column permutation: PSUM partition m = t*64 + h*32 + p'
    # holds feature f = h*64 + t*32 + p' (enables 64-partition shuffle DMAs)
    mm = np.arange(128)
    perm = (((mm % 64) // 32) * 64 + (mm // 64) * 32 + mm % 32)

    in_maps = []
    for c in range(NCORES):
        sl = slice(c * FC, (c + 1) * FC)
        in_maps.append({
            "xq8_nd": xq8,
            "xk8_nd": xk8,
            "wq8": np.ascontiguousarray(
                (Wq[sl][perm].T * W8_SCALE).astype(FP8_NP)),
            "wk8": np.ascontiguousarray(
                (Wk[sl][perm].T * W8_SCALE).astype(FP8_NP)),
            "wvt": np.ascontiguousarray(Wv[sl].T.astype(FP8_NP)),
            # blocked: [p, (n, g, f)] where row = n*l + g*128 + p
            "q_res": np.ascontiguousarray(
                qf[:, sl].astype(BF16_NP).reshape(n, l // 128, 128, FC)
                .transpose(2, 0, 1, 3).reshape(128, -1)),
            "gamma": np.ascontiguousarray(gamma[sl].reshape(1, FC)),
            "beta": np.ascontiguousarray(beta[sl].reshape(1, FC)),
        })
    return in_maps


def kernel(**inputs):
    l = np.asarray(inputs["query"]).shape[1]
    if "nc" not in _cached or _cached.get("l") != l:
        _cached["nc"] = build_program(l)
        _cached["l"] = l
    nc = _cached["nc"]

    in_maps = make_in_maps(inputs, l)
    run_np = get_runner(nc)[0]
    results = run_np(in_maps)

    n = np.asarray(inputs["query"]).shape[0]
    out = np.zeros((n, l, D), dtype=np.float32)
    for c in range(NCORES):
        sl = slice(c * FC, (c + 1) * FC)
        arr = results[c]["out_s"].reshape(128, n, l // 128, FC)
        out[:, :, sl] = arr.transpose(1, 2, 0, 3).reshape(n, l, FC).astype(
            np.float32)
    return out
